# revision 1
# baseline (speedup 1.0000x reference)
"""DiscoNetFusion Trainium2 kernel (8 NeuronCores, SPMD).

Strategy
--------
Only ego agent i=0 of each scene contributes to the output, so per scene b we
need the L_b = record_len[b] neighbor warps nbr[b,0,j], the 4-layer 1x1-conv
attention head on z=[nbr;ego], a softmax over j, and the weighted feature sum
followed by a channel MLP.

Core k handles output rows [10k, 10k+10) of ALL scenes (8 cores x 10 rows =
80 rows).  Per core there are sum(record_len)=9 (scene, agent) units; each
unit is 1600 output pixels (padded to 1664 = 13 tiles of 128).

Bilinear warp = DMA gather (pixel-major dup-row fp16 source; one index
fetches the 2x2 tap patch: entries (y0,x0),(y0,x0+1), each entry holding
rows y0 and y0+1 of all 64 channels) + a lerp combine on the vector engine
with per-partition (=per-pixel) scalars, then a PE transpose back to
channel-major for the conv stack.  Softmax/attention is computed with exp on
the scalar engine, j-reduction + partition-broadcasts via tiny PE matmuls.

Host side (numpy) only prepares gather indices / lerp weights / warped-mask
maps (data-independent of x) and reassembles the 8 row-blocks.
"""

import dataclasses
import os

import numpy as np

import concourse.bacc as bacc
import concourse.mybir as mybir
from concourse.bass_utils import run_bass_kernel_spmd
from concourse.tile import TileContext

F32 = mybir.dt.float32
F16 = mybir.dt.float16
I16 = mybir.dt.int16
Alu = mybir.AluOpType
Act = mybir.ActivationFunctionType

C = 64
H = 80
W = 160
B = 3
L = 4
EPS = 1e-5
NCORES = 8
R = H // NCORES            # output rows per core
PX = R * W                 # 1600 real pixels
NT = 13                    # px tiles of 128
PXP = NT * 128             # 1664 padded pixels
NENT = H * W               # gather source entries per agent
CHUNKS = [(0, 512), (512, 512), (1024, 512), (1536, 128)]
HCHUNKS = [(0, 832, [(0, 512), (512, 320)]), (832, 832, [(0, 512), (512, 320)])]
OUT_CHUNKS = [(0, 512), (512, 512), (1024, 512), (1536, 64)]


def _wrap_idx(idx_flat):
    """[N] -> [128, N//16] wrapped-in-16-partitions, replicated to 8 groups."""
    n = idx_flat.shape[0]
    w = idx_flat.reshape(n // 16, 16).T  # [16, N//16]
    return np.tile(w, (8, 1)).astype(np.int16)


def _host_warp_prep(theta, h0):
    """Per-(unit) gather indices + lerp scalars for output rows [h0,h0+R).

    Returns idx[PXP] int32 (entry units), fx, c0, c1 [PXP] f32 and
    (y0,x0,fy,fx,scale) pieces needed for the mask warp.
    """
    ys = np.linspace(-1.0, 1.0, H, dtype=np.float32)[h0 : h0 + R]
    xs = np.linspace(-1.0, 1.0, W, dtype=np.float32)
    gx, gy = np.meshgrid(xs, ys)  # [R, W]
    sx = theta[0, 0] * gx + theta[0, 1] * gy + theta[0, 2]
    sy = theta[1, 0] * gx + theta[1, 1] * gy + theta[1, 2]
    px = (sx + 1.0) * (W - 1) / 2.0
    py = (sy + 1.0) * (H - 1) / 2.0
    x0 = np.floor(px).astype(np.int64)
    y0 = np.floor(py).astype(np.int64)
    fx = (px - x0).astype(np.float32)
    fy = (py - y0).astype(np.float32)

    scale = np.ones_like(fx)
    # x handling
    x0c = np.clip(x0, 0, W - 1)
    fxp = fx.copy()
    m = x0 == W - 1          # x1 out of bounds -> drop B/D taps
    fxp[m] = 0.0
    scale[m] *= 1.0 - fx[m]
    m = x0 == -1             # x0 out of bounds -> entry at x=0 is the B tap
    x0c[m] = 0
    fxp[m] = 0.0
    scale[m] *= fx[m]
    m = (x0 < -1) | (x0 > W - 1)
    x0c[m] = 0
    fxp[m] = 0.0
    scale[m] = 0.0
    # y handling (entry [y0] holds rows y0,y0+1; row 80 half is zeros)
    y0c = np.clip(y0, 0, H - 1)
    fyp = fy.copy()
    m = y0 == -1             # row0 is the F tap
    y0c[m] = 0
    fyp[m] = 0.0
    scale[m] *= fy[m]
    m = (y0 < -1) | (y0 > H - 1)
    y0c[m] = 0
    fyp[m] = 0.0
    scale[m] = 0.0

    idx = (y0c * W + x0c).reshape(-1)
    c0 = (scale * (1.0 - fyp)).reshape(-1)
    c1 = (scale * fyp).reshape(-1)
    fxp = fxp.reshape(-1)

    pad = PXP - PX
    idx = np.concatenate([idx, np.zeros(pad, np.int64)])
    fxp = np.concatenate([fxp, np.zeros(pad, np.float32)])
    c0 = np.concatenate([c0, np.zeros(pad, np.float32)])
    c1 = np.concatenate([c1, np.zeros(pad, np.float32)])
    return idx, fxp, c0, c1


def _host_warp_mask(mask_bj, theta, h0):
    """Bilinear warp of one [H,W] mask (zero padding) for rows [h0,h0+R)."""
    ys = np.linspace(-1.0, 1.0, H, dtype=np.float32)[h0 : h0 + R]
    xs = np.linspace(-1.0, 1.0, W, dtype=np.float32)
    gx, gy = np.meshgrid(xs, ys)
    sx = theta[0, 0] * gx + theta[0, 1] * gy + theta[0, 2]
    sy = theta[1, 0] * gx + theta[1, 1] * gy + theta[1, 2]
    px = (sx + 1.0) * (W - 1) / 2.0
    py = (sy + 1.0) * (H - 1) / 2.0
    x0 = np.floor(px).astype(np.int64)
    y0 = np.floor(py).astype(np.int64)
    wx = (px - x0).astype(np.float32)
    wy = (py - y0).astype(np.float32)

    def gat(xi, yi):
        inb = ((xi >= 0) & (xi < W) & (yi >= 0) & (yi < H)).astype(np.float32)
        v = mask_bj[np.clip(yi, 0, H - 1), np.clip(xi, 0, W - 1)]
        return v * inb

    out = (
        gat(x0, y0) * (1 - wx) * (1 - wy)
        + gat(x0 + 1, y0) * wx * (1 - wy)
        + gat(x0, y0 + 1) * (1 - wx) * wy
        + gat(x0 + 1, y0 + 1) * wx * wy
    )
    return out.reshape(-1)  # [PX]


class _StageDone(Exception):
    pass


def _build_program(nagents, scene_of, src_names):
    """Build the SPMD Bass program (identical for all cores)."""
    nc = bacc.Bacc("TRN2", target_bir_lowering=False, num_devices=NCORES)

    srcs = [
        nc.dram_tensor(nm, [NENT + 1, 2 * C], F16, kind="ExternalInput")
        for nm in src_names
    ]
    idx_all = nc.dram_tensor("idx_all", [128, nagents * (PXP // 16)], I16,
                             kind="ExternalInput")
    scal_all = nc.dram_tensor("scal_all", [128, nagents * 8 * NT], F16,
                              kind="ExternalInput")
    ego_all = nc.dram_tensor("ego_all", [C, B * PXP], F16, kind="ExternalInput")
    cm_all = nc.dram_tensor("cm_all", [nagents, 2 * PXP], F16,
                            kind="ExternalInput")
    w1 = nc.dram_tensor("w1", [2 * C, 2 * C], F16, kind="ExternalInput")
    w2 = nc.dram_tensor("w2", [2 * C, 32], F16, kind="ExternalInput")
    w3 = nc.dram_tensor("w3", [32, 32], F16, kind="ExternalInput")
    w4 = nc.dram_tensor("w4", [8, 32], F16, kind="ExternalInput")
    mlpw = nc.dram_tensor("mlpw", [C, C], F16, kind="ExternalInput")
    # per-partition scale/bias vectors: [128, 6] f32
    #   col0: a1, col1: b1, col2: a2, col3: b2, col4: a3, col5: b3
    sb = nc.dram_tensor("sb", [128, 6], F32, kind="ExternalInput")
    cb4b = nc.dram_tensor("cb4b", [65, 1], F32, kind="ExternalInput")
    sb2 = nc.dram_tensor("sb2", [96, 1], F32, kind="ExternalInput")
    sb3 = nc.dram_tensor("sb3", [72, 1], F32, kind="ExternalInput")
    mlpb = nc.dram_tensor("mlpb", [C, 1], F32, kind="ExternalInput")
    ident = nc.dram_tensor("ident", [128, 128], F16, kind="ExternalInput")
    ones64 = nc.dram_tensor("ones64", [1, C], F16, kind="ExternalInput")
    ind_js = nc.dram_tensor("ind_js", [nagents, B], F16, kind="ExternalInput")
    ind_sj = nc.dram_tensor("ind_sj", [B, nagents], F16, kind="ExternalInput")
    npair = (nagents + 1) // 2
    pairsel = nc.dram_tensor("pairsel", [nagents, npair * 128], F16,
                             kind="ExternalInput")
    out = nc.dram_tensor("out", [B * C, PX], F32, kind="ExternalOutput")
    debug = bool(os.environ.get("KERNEL_DEBUG"))
    if debug:
        dbg_s = nc.dram_tensor("dbg_s", [16, PXP], F16, kind="ExternalOutput")
        dbg_z = nc.dram_tensor("dbg_z", [128, PXP], F16, kind="ExternalOutput")
        dbg_alp = nc.dram_tensor("dbg_alp", [16, PXP], F16, kind="ExternalOutput")
        dbg_u = nc.dram_tensor("dbg_u", [C, PXP], F16, kind="ExternalOutput")
        dbg_h1 = nc.dram_tensor("dbg_h1", [128, 512], F16, kind="ExternalOutput")
        dbg_h2 = nc.dram_tensor("dbg_h2", [32, 512], F16, kind="ExternalOutput")
        dbg_h3 = nc.dram_tensor("dbg_h3", [8, 512], F16, kind="ExternalOutput")

    stage = int(os.environ.get("KERNEL_STAGE", "3"))
    with TileContext(nc) as tc:
        with (
            tc.tile_pool(name="const", bufs=1) as cpool,
            tc.tile_pool(name="zs", bufs=1) as zpool,
            tc.tile_pool(name="work", bufs=2) as wpool,
            tc.tile_pool(name="att", bufs=1) as apool,
            tc.tile_pool(name="pmm", bufs=1, space="PSUM") as pmm,
            tc.tile_pool(name="ptr", bufs=2, space="PSUM") as ptr,
        ):
            # ---- constants ----
            idx_t = cpool.tile([128, nagents * (PXP // 16)], I16)
            nc.sync.dma_start(idx_t[:], idx_all[:, :])
            scal_t = cpool.tile([128, nagents * 8 * NT], F16)
            nc.sync.dma_start(scal_t[:], scal_all[:, :])
            ego_t = cpool.tile([C, B * PXP], F16)
            nc.sync.dma_start(ego_t[:], ego_all[:, :])
            cm_t = cpool.tile([nagents, 2 * PXP], F16)
            nc.sync.dma_start(cm_t[:], cm_all[:, :])
            w1_t = cpool.tile([2 * C, 2 * C], F16)
            nc.sync.dma_start(w1_t[:], w1[:, :])
            w2_t = cpool.tile([2 * C, 32], F16)
            nc.sync.dma_start(w2_t[:], w2[:, :])
            w3_t = cpool.tile([128, 32], F16)
            w4_t = cpool.tile([128, 32], F16)
            for q in range(3):
                nc.sync.dma_start(w3_t[32 * q : 32 * q + 32, :], w3[:, :])
                nc.sync.dma_start(w4_t[32 * q : 32 * q + 8, :], w4[:, :])
            mlpw_t = cpool.tile([C, C], F16)
            nc.sync.dma_start(mlpw_t[:], mlpw[:, :])
            sb_t = cpool.tile([128, 6], F32)
            nc.sync.dma_start(sb_t[:], sb[:, :])
            cb4_t = cpool.tile([65, 1], F32)
            nc.sync.dma_start(cb4_t[:], cb4b[:, :])
            mlpb_t = cpool.tile([C, 1], F32)
            nc.sync.dma_start(mlpb_t[:], mlpb[:, :])
            id_t = cpool.tile([128, 128], F16)
            nc.sync.dma_start(id_t[:], ident[:, :])
            ones_t = cpool.tile([1, C], F16)
            nc.sync.dma_start(ones_t[:], ones64[:, :])
            indjs_t = cpool.tile([nagents, B], F16)
            nc.sync.dma_start(indjs_t[:], ind_js[:, :])
            indsj_t = cpool.tile([B, nagents], F16)
            nc.sync.dma_start(indsj_t[:], ind_sj[:, :])
            pairsel_t = cpool.tile([nagents, npair * 128], F16)
            nc.sync.dma_start(pairsel_t[:], pairsel[:, :])

            z_all = [zpool.tile([128, PXP], F16, name=f"z{j}", tag=f"z{j}")
                     for j in range(nagents)]
            h1_all = [None, None, None]
            sb2_t = cpool.tile([96, 1], F32)
            nc.sync.dma_start(sb2_t[:], sb2[:, :])
            sb3_t = cpool.tile([72, 1], F32)
            nc.sync.dma_start(sb3_t[:], sb3[:, :])
            s_all = apool.tile([nagents, PXP], F16)

            for j in range(nagents):
                b = scene_of[j]
                # ---- gather 2x2 taps, pixel-major ----
                g_t = wpool.tile([128, NT, 4 * C], F16, tag="g", bufs=3)
                src_flat = srcs[j][:, :].rearrange("a b -> (a b)")
                src_win = dataclasses.replace(
                    src_flat, ap=[[2 * C, NENT], [1, 4 * C]]
                )
                for (gt0, gtn) in ((0, 7), (7, 6)):
                    nc.gpsimd.dma_gather(
                        g_t[:, gt0 : gt0 + gtn, :],
                        src_win,
                        idx_t[:, j * (PXP // 16) + gt0 * 8 :
                              j * (PXP // 16) + (gt0 + gtn) * 8],
                        num_idxs=gtn * 128,
                        num_idxs_reg=gtn * 128,
                        elem_size=4 * C,
                        elem_step=2 * C,
                        single_packet=False,
                    )
                # ---- bilinear combine: nbr = w00*A+w01*B + w10*C+w11*D ----
                # weights live packed [128, NT, 4] (w00,w10,w01,w11); read with
                # free-step-0 APs to broadcast each weight over 64 channels.
                t1_t = wpool.tile([128, NT, 2 * C], F16, tag="t1", bufs=3)
                t2_t = wpool.tile([128, NT, 2 * C], F16, tag="t2", bufs=3)
                nbr_t = wpool.tile([128, NT, C], F16, tag="nbr", bufs=3)
                # weights stored duplicated in pairs: col 8t+2q+d = w_q[tile t]
                # one mult per tap block keeps free dims at 3 (walrus limit)
                # while the packed [1,2] last dim preserves the DVE 2x mode
                wq = scal_t[:, j * 8 * NT : (j + 1) * 8 * NT]
                for q, dst in ((0, t1_t[:, :, 0:C]), (1, t1_t[:, :, C : 2 * C]),
                               (2, t2_t[:, :, 0:C]), (3, t2_t[:, :, C : 2 * C])):
                    w_ap = dataclasses.replace(
                        wq, offset=wq.offset + 2 * q,
                        ap=[list(wq.ap[0]), [8, NT], [0, C // 2], [1, 2]])
                    src = g_t[:, :, q * C : (q + 1) * C]
                    nc.vector.tensor_tensor(
                        dst.rearrange("p a (c d) -> p a c d", d=2),
                        src.rearrange("p a (c d) -> p a c d", d=2),
                        w_ap, Alu.mult)
                nc.vector.tensor_tensor(t1_t[:, :, 0:C], t1_t[:, :, 0:C],
                                        t2_t[:, :, 0:C], Alu.add)
                nc.vector.tensor_tensor(t1_t[:, :, C : 2 * C],
                                        t1_t[:, :, C : 2 * C],
                                        t2_t[:, :, C : 2 * C], Alu.add)
                nc.vector.tensor_tensor(
                    nbr_t[:], t1_t[:, :, 0:C], t1_t[:, :, C : 2 * C], Alu.add)
                # ---- transpose px-major -> channel-major into z ----
                # 4 transposes land at column offsets of one [64, 512] psum
                # bank; a single evac moves all 4 (alternating ACT / DVE)
                z_t = z_all[j]
                for t0 in range(0, NT, 4):
                    tn = min(4, NT - t0)
                    tr_ps = ptr.tile([C, 512], F16, tag="tr")
                    for t in range(t0, t0 + tn):
                        nc.tensor.transpose(
                            tr_ps[:, 128 * (t - t0) : 128 * (t - t0 + 1)],
                            nbr_t[:, t, :], id_t[:])
                    dst = z_t[0:C, 128 * t0 : 128 * (t0 + tn)]
                    nc.scalar.activation(dst, tr_ps[:, 0 : 128 * tn],
                                         Act.Copy)
                # ego half
                nc.sync.dma_start(
                    z_t[C : 2 * C, :], ego_t[:, b * PXP : (b + 1) * PXP]
                )
                # ---- conv1 + h1 (per agent; trio stages run below) ----
                if stage < 2:
                    continue
                h1_j = wpool.tile([128, PXP], F16, name=f"h1_{j}", tag=f"h1_{j % 3}")
                h1_all[j % 3] = h1_j
                for (o, n, mms) in HCHUNKS:
                    p1 = pmm.tile([128, 832], F32, tag="p1", bufs=1)
                    for (mo, mn) in mms:
                        nc.tensor.matmul(p1[:, mo : mo + mn], w1_t[:],
                                         z_t[:, o + mo : o + mo + mn],
                                         start=True, stop=True)
                    nc.scalar.activation(h1_j[:, o : o + n], p1[:, 0:n],
                                         Act.Relu, bias=sb_t[:, 1:2], scale=1.0)

                # ---- conv2..4 for a completed trio of agents ----
                if j % 3 == 2 or j == nagents - 1:
                    trio = [jj for jj in (j - j % 3 + q for q in range(3))
                            if jj <= j]
                    hs2 = wpool.tile([96, PXP], F16, tag="hs2")
                    hs3 = wpool.tile([72, PXP], F16, tag="hs3")
                    srow = wpool.tile([65, PXP], F16, tag="srow")
                    for (o, n, mms) in HCHUNKS:
                        sl = slice(o, o + n)
                        ph2 = pmm.tile([96, 832], F32, tag="p34", bufs=2)
                        for q, jj in enumerate(trio):
                            for (mo, mn) in mms:
                                nc.tensor.matmul(
                                    ph2[32 * q : 32 * q + 32, mo : mo + mn],
                                    w2_t[:],
                                    h1_all[q][:, o + mo : o + mo + mn],
                                    start=True, stop=True)
                        nc.scalar.activation(hs2[0 : 32 * len(trio), sl],
                                             ph2[0 : 32 * len(trio), 0:n],
                                             Act.Relu,
                                             bias=sb2_t[0 : 32 * len(trio), 0:1],
                                             scale=1.0)
                        p34 = pmm.tile([96, 832], F32, tag="p34", bufs=2)
                        for q, jj in enumerate(trio):
                            for (mo, mn) in mms:
                                nc.tensor.matmul(
                                    p34[32 * q : 32 * q + 32, mo : mo + mn],
                                    w3_t[32 * q : 32 * q + 32, :],
                                    hs2[32 * q : 32 * q + 32,
                                        o + mo : o + mo + mn],
                                    start=True, stop=True)
                        nc.scalar.activation(
                            hs3[0 : 32 * (len(trio) - 1) + 8, sl],
                            p34[0 : 32 * (len(trio) - 1) + 8, 0:n], Act.Relu,
                            bias=sb3_t[0 : 32 * (len(trio) - 1) + 8, 0:1],
                            scale=1.0)
                        p4 = pmm.tile([96, 832], F32, tag="p34", bufs=2)
                        for q, jj in enumerate(trio):
                            for (mo, mn) in mms:
                                nc.tensor.matmul(
                                    p4[32 * q : 32 * q + 32, mo : mo + mn],
                                    w4_t[32 * q : 32 * q + 8, :],
                                    hs3[32 * q : 32 * q + 8,
                                        o + mo : o + mo + mn],
                                    start=True, stop=True)
                        nc.scalar.activation(srow[0 : 32 * (len(trio) - 1) + 1, sl],
                                             p4[0 : 32 * (len(trio) - 1) + 1, 0:n],
                                             Act.Relu,
                                             bias=cb4_t[0 : 32 * (len(trio) - 1) + 1, 0:1],
                                             scale=1.0)
                    for q, jj in enumerate(trio):
                        nc.sync.dma_start(s_all[jj : jj + 1, :],
                                          srow[32 * q : 32 * q + 1, :])

            # ---- softmax over j (unnormalized exp; NEG-masked via cm==0) ----
            if stage < 3:
                if debug:
                    nc.sync.dma_start(dbg_z[:, :], z_all[0][:])
                    if stage >= 2:
                        nc.sync.dma_start(dbg_s[0:nagents, :], s_all[:])
                do_attention = False
            else:
                do_attention = True
            if do_attention:
                e_t = apool.tile([nagents, PXP], F16)
                nc.scalar.activation(e_t[:], s_all[:], Act.Exp)
                ep_t = apool.tile([nagents, PXP], F16)   # e * (cm != 0)
                nc.vector.tensor_tensor(ep_t[:], e_t[:], cm_t[:, PXP : 2 * PXP],
                                        Alu.mult)
                al_t = apool.tile([nagents, PXP], F16)   # alpha = e * cm
                nc.vector.tensor_tensor(al_t[:], e_t[:], cm_t[:, 0:PXP], Alu.mult)
                # den per scene + reciprocal + broadcast back to agents
                rec_t = apool.tile([B, PXP], F16)
                alp_t = apool.tile([nagents, PXP], F16)  # alpha / den
                for (o, n) in CHUNKS:
                    sl = slice(o, o + n)
                    dps = pmm.tile([B, 512], F32, tag="p34", bufs=2)
                    nc.tensor.matmul(dps[:, 0:n], indjs_t[:], ep_t[:, sl],
                                     start=True, stop=True)
                    with nc.allow_low_precision(reason="den>=1, fp16 rec ok"):
                        nc.vector.reciprocal(rec_t[:, sl], dps[:, 0:n])
                    rps = pmm.tile([nagents, 512], F32, tag="p34", bufs=2)
                    nc.tensor.matmul(rps[:, 0:n], indsj_t[:], rec_t[:, sl],
                                     start=True, stop=True)
                    nc.vector.tensor_tensor(alp_t[:, sl], al_t[:, sl], rps[:, 0:n],
                                            Alu.mult)

                # ---- weighted sum over agents, per scene ----
                u_all = [apool.tile([C, PXP], F16, name=f"u{b}", tag=f"u{b}")
                         for b in range(B)]
                first = {(b, o): True for b in range(B) for (o, n, _) in HCHUNKS}
                seen = set()
                for p in range(npair):
                    pj = [j for j in (2 * p, 2 * p + 1) if j < nagents]
                    for (o, n, mms) in HCHUNKS:
                        sl = slice(o, o + n)
                        abps = pmm.tile([128, 832], F32, tag="p1", bufs=1)
                        for (mo, mn) in mms:
                            nc.tensor.matmul(abps[:, mo : mo + mn],
                                             pairsel_t[:, 128 * p : 128 * (p + 1)],
                                             alp_t[:, o + mo : o + mo + mn],
                                             start=True, stop=True)
                        for ii, j in enumerate(pj):
                            b = scene_of[j]
                            half = abps[64 * ii : 64 * (ii + 1), 0:n]
                            if (j, o) not in seen:
                                seen.add((j, o))
                                if first[(b, o)]:
                                    first[(b, o)] = False
                                    nc.vector.tensor_tensor(
                                        u_all[b][:, sl], z_all[j][0:C, sl],
                                        half, Alu.mult,
                                    )
                                    continue
                            pr = wpool.tile([C, 832], F16, tag="pr")
                            nc.vector.tensor_tensor(
                                pr[:, 0:n], z_all[j][0:C, sl], half, Alu.mult
                            )
                            nc.vector.tensor_tensor(
                                u_all[b][:, sl], u_all[b][:, sl], pr[:, 0:n],
                                Alu.add,
                            )

                if debug:
                    nc.sync.dma_start(dbg_s[0:nagents, :], s_all[:])
                    nc.sync.dma_start(dbg_z[:, :], z_all[0][:])
                    nc.sync.dma_start(dbg_alp[0:nagents, :], alp_t[:])
                    nc.sync.dma_start(dbg_u[:, :], u_all[0][:])
                # ---- MLP + bias, write out ----
                for b in range(B):
                    for (o, n, mms) in [(0, 832, [(0, 512), (512, 320)]),
                                        (832, 768, [(0, 512), (512, 256)])]:
                        mps = pmm.tile([C, 832], F32, tag="p34", bufs=2)
                        for (mo, mn) in mms:
                            nc.tensor.matmul(mps[:, mo : mo + mn], mlpw_t[:],
                                             u_all[b][:, o + mo : o + mo + mn],
                                             start=True, stop=True)
                        ob = wpool.tile([C, 832], F32, tag="ob")
                        nc.scalar.activation(ob[:, 0:n], mps[:, 0:n],
                                             Act.Identity, bias=mlpb_t[:, 0:1],
                                             scale=1.0)
                        nc.sync.dma_start(out[b * C : (b + 1) * C, o : o + n],
                                          ob[:, 0:n])

    nc.compile()
    return nc


_PROG_CACHE = {}
_LAST_RES = None


def kernel(**inputs):
    x = np.asarray(inputs["x"], np.float32)
    mask = np.asarray(inputs["mask"], np.float32)
    record_len = np.asarray(inputs["record_len"])
    ptm = np.asarray(inputs["pairwise_t_matrix"], np.float32)
    rec = [int(v) for v in record_len]
    agents = [(b, j) for b in range(B) for j in range(rec[b])]
    nagents = len(agents)
    scene_of = [b for (b, j) in agents]

    # ---- regroup x into per-scene node features ----
    node = np.zeros((B, L, C, H, W), np.float32)
    idx0 = 0
    for b, n in enumerate(rec):
        node[b, :n] = x[idx0 : idx0 + n]
        idx0 += n

    # ---- gather sources: dup-row pixel-major fp16 ----
    src_names = [f"src{a}" for a in range(nagents)]
    src_arrs = {}
    for a, (b, j) in enumerate(agents):
        feat = node[b, j]  # [C, H, W]
        ent = np.zeros((H + 1, W, 2 * C), np.float16)
        pm = feat.transpose(1, 2, 0).astype(np.float16)  # [H, W, C]
        ent[:H, :, :C] = pm
        ent[:H - 1, :, C:] = pm[1:]
        # row H-1 second half stays zero (virtual row 80 = 0)
        arr = np.zeros((NENT + 1, 2 * C), np.float16)
        arr[:NENT] = ent[:H].reshape(NENT, 2 * C)
        src_arrs[src_names[a]] = arr

    # ---- per-core index/scalar/mask/ego prep ----
    per_core = []
    for k in range(NCORES):
        h0 = k * R
        idx_cols = np.zeros((128, nagents * (PXP // 16)), np.int16)
        scal_cols = np.zeros((128, nagents * 8 * NT), np.float16)
        cm_arr = np.zeros((nagents, 2 * PXP), np.float16)
        ego_arr = np.zeros((C, B * PXP), np.float16)
        for b in range(B):
            ego = node[b, 0][:, h0 : h0 + R, :].reshape(C, PX)
            ego_arr[:, b * PXP : b * PXP + PX] = ego.astype(np.float16)
        for a, (b, j) in enumerate(agents):
            theta = ptm[b, j, 0]  # theta[b, i=0, j] = ptm[b, j, 0]
            idx, fxp, c0, c1 = _host_warp_prep(theta, h0)
            idx_cols[:, a * (PXP // 16) : (a + 1) * (PXP // 16)] = _wrap_idx(idx)
            w00 = (c0 * (1.0 - fxp)).astype(np.float16)
            w10 = (c1 * (1.0 - fxp)).astype(np.float16)
            w01 = (c0 * fxp).astype(np.float16)
            w11 = (c1 * fxp).astype(np.float16)
            sc = scal_cols[:, a * 8 * NT : (a + 1) * 8 * NT]
            for t in range(NT):
                pxs = slice(128 * t, 128 * (t + 1))
                for q, wv in enumerate((w00, w10, w01, w11)):
                    sc[:, 8 * t + 2 * q] = wv[pxs]
                    sc[:, 8 * t + 2 * q + 1] = wv[pxs]
            wm = _host_warp_mask(mask[b, j], theta, h0)
            cm_arr[a, :PX] = wm.astype(np.float16)
            cm_arr[a, PXP : PXP + PX] = (wm != 0).astype(np.float16)
            cm_arr[a, PXP + PX :] = 1.0
        per_core.append((idx_cols, scal_cols, cm_arr, ego_arr))

    # ---- shared small tensors ----
    def gf(n):
        return np.asarray(inputs[n], np.float32)

    sb = np.zeros((128, 6), np.float32)
    sb2v = np.zeros((96, 1), np.float32)
    sb3v = np.zeros((72, 1), np.float32)
    a1 = gf("g1") / np.sqrt(gf("rv1") + EPS)
    sb[:, 1] = gf("be1") + (gf("cb1") - gf("rm1")) * a1
    a2 = gf("g2") / np.sqrt(gf("rv2") + EPS)
    b2f = gf("be2") + (gf("cb2") - gf("rm2")) * a2
    a3 = gf("g3") / np.sqrt(gf("rv3") + EPS)
    b3f = gf("be3") + (gf("cb3") - gf("rm3")) * a3
    for q in range(3):
        sb2v[32 * q : 32 * q + 32, 0] = b2f
        sb3v[32 * q : 32 * q + 8, 0] = b3f

    ind_js = np.zeros((nagents, B), np.float16)
    for a, bb in enumerate(scene_of):
        ind_js[a, bb] = 1.0
    npair = (nagents + 1) // 2
    psel = np.zeros((nagents, npair * 128), np.float16)
    for p in range(npair):
        psel[2 * p, 128 * p : 128 * p + 64] = 1.0
        if 2 * p + 1 < nagents:
            psel[2 * p + 1, 128 * p + 64 : 128 * (p + 1)] = 1.0
    shared = {
        "idx_all": None,  # per core
        "pairsel": psel,
        "w1": (gf("w1") * a1[None, :]).astype(np.float16),
        "w2": (gf("w2") * a2[None, :]).astype(np.float16),
        "w3": np.pad((gf("w3") * a3[None, :]).astype(np.float16),
                     ((0, 0), (0, 24))),
        "w4": np.pad(gf("w4").astype(np.float16), ((0, 0), (0, 31))),
        "mlpw": gf("mlp_w").astype(np.float16),
        "sb": sb,
        "sb2": sb2v,
        "sb3": sb3v,
        "cb4b": np.full((65, 1), gf("cb4")[0], np.float32),
        "mlpb": gf("mlp_b").reshape(C, 1),
        "ident": np.eye(128, dtype=np.float16),
        "ones64": np.ones((1, C), np.float16),
        "ind_js": ind_js,
        "ind_sj": ind_js.T.copy(),
    }
    shared.update(src_arrs)
    del shared["idx_all"]

    key = (nagents, tuple(scene_of))
    if key not in _PROG_CACHE:
        _PROG_CACHE[key] = _build_program(nagents, scene_of, src_names)
    nc = _PROG_CACHE[key]

    in_maps = []
    for k in range(NCORES):
        idx_cols, scal_cols, cm_arr, ego_arr = per_core[k]
        m = dict(shared)
        m["idx_all"] = idx_cols
        m["scal_all"] = scal_cols
        m["cm_all"] = cm_arr
        m["ego_all"] = ego_arr
        in_maps.append(m)

    trace = bool(os.environ.get("KERNEL_TRACE"))
    res = run_bass_kernel_spmd(nc, in_maps, core_ids=list(range(NCORES)),
                               trace=trace)
    global _LAST_RES
    _LAST_RES = res

    out = np.zeros((B, C, H, W), np.float32)
    for k in range(NCORES):
        o = res.results[k]["out"]  # [B*C, PX]
        out[:, :, k * R : (k + 1) * R, :] = o.reshape(B, C, R, W)
    return out



# revision 5
# speedup vs baseline: 1.1187x; 1.1187x over previous
"""DiscoNetFusion Trainium2 kernel (8 NeuronCores, SPMD).

Strategy
--------
Only ego agent i=0 of each scene contributes to the output, so per scene b we
need the L_b = record_len[b] neighbor warps nbr[b,0,j], the 4-layer 1x1-conv
attention head on z=[nbr;ego], a softmax over j, and the weighted feature sum
followed by a channel MLP.

Core k handles output rows [10k, 10k+10) of ALL scenes (8 cores x 10 rows =
80 rows).  Per core there are sum(record_len)=9 (scene, agent) units; each
unit is 1600 output pixels (padded to 1664 = 13 tiles of 128).

Bilinear warp = DMA gather (pixel-major dup-row fp16 source; one index
fetches the 2x2 tap patch) + a lerp combine on the vector engine with
per-partition (=per-pixel) scalars, then a PE transpose back to channel-major
for the conv stack.

Attention runs in PIXEL-major: s [9, px] is transposed by tiny PE matmuls to
[px-partition, tile, agent], so softmax/normalize ops shrink from
[9, 1664]-sized (927ns) to [128, 13, 9]-sized (~100-500ns) DVE ops.  The
weighted sum alpha*nbr reuses the pre-transpose pixel-major nbr tiles, gets
folded per scene with a few adds, and is transposed back to channel-major by
PE for the MLP.  The MLP bias rides as a 65th weight row against a constant
ones row in the moving operand, and the result is DMA'd to HBM directly from
PSUM.
"""

import dataclasses
import os

import numpy as np

import concourse.bacc as bacc
import concourse.mybir as mybir
from concourse.bass_utils import run_bass_kernel_spmd
from concourse.tile import TileContext

F32 = mybir.dt.float32
F16 = mybir.dt.float16
I16 = mybir.dt.int16
Alu = mybir.AluOpType
Act = mybir.ActivationFunctionType

C = 64
H = 80
W = 160
B = 3
L = 4
EPS = 1e-5
NCORES = 8
R = H // NCORES            # output rows per core
PX = R * W                 # 1600 real pixels
NT = 13                    # px tiles of 128
PXP = NT * 128             # 1664 padded pixels
NENT = H * W               # gather source entries per agent
HCHUNKS = [(0, 832, [(0, 512), (512, 320)]), (832, 832, [(0, 512), (512, 320)])]
# u transposes write 128-wide blocks; chunks must be tile-aligned
UCHUNKS = [(0, 512), (512, 512), (1024, 512), (1536, 128)]
MCHUNKS = [(0, 832, [(0, 512), (512, 320)]), (832, 768, [(0, 512), (512, 256)])]


def _wrap_idx(idx_flat):
    """[N] -> [128, N//16] wrapped-in-16-partitions, replicated to 8 groups."""
    n = idx_flat.shape[0]
    w = idx_flat.reshape(n // 16, 16).T  # [16, N//16]
    return np.tile(w, (8, 1)).astype(np.int16)


def _host_warp_prep(theta, h0):
    """Per-(unit) gather indices + lerp scalars for output rows [h0,h0+R).

    Returns idx[PXP] int32 (entry units), fx, c0, c1 [PXP] f32.
    """
    ys = np.linspace(-1.0, 1.0, H, dtype=np.float32)[h0 : h0 + R]
    xs = np.linspace(-1.0, 1.0, W, dtype=np.float32)
    gx, gy = np.meshgrid(xs, ys)  # [R, W]
    sx = theta[0, 0] * gx + theta[0, 1] * gy + theta[0, 2]
    sy = theta[1, 0] * gx + theta[1, 1] * gy + theta[1, 2]
    px = (sx + 1.0) * (W - 1) / 2.0
    py = (sy + 1.0) * (H - 1) / 2.0
    x0 = np.floor(px).astype(np.int64)
    y0 = np.floor(py).astype(np.int64)
    fx = (px - x0).astype(np.float32)
    fy = (py - y0).astype(np.float32)

    scale = np.ones_like(fx)
    # x handling
    x0c = np.clip(x0, 0, W - 1)
    fxp = fx.copy()
    m = x0 == W - 1          # x1 out of bounds -> drop B/D taps
    fxp[m] = 0.0
    scale[m] *= 1.0 - fx[m]
    m = x0 == -1             # x0 out of bounds -> entry at x=0 is the B tap
    x0c[m] = 0
    fxp[m] = 0.0
    scale[m] *= fx[m]
    m = (x0 < -1) | (x0 > W - 1)
    x0c[m] = 0
    fxp[m] = 0.0
    scale[m] = 0.0
    # y handling (entry [y0] holds rows y0,y0+1; row 80 half is zeros)
    y0c = np.clip(y0, 0, H - 1)
    fyp = fy.copy()
    m = y0 == -1             # row0 is the F tap
    y0c[m] = 0
    fyp[m] = 0.0
    scale[m] *= fy[m]
    m = (y0 < -1) | (y0 > H - 1)
    y0c[m] = 0
    fyp[m] = 0.0
    scale[m] = 0.0

    idx = (y0c * W + x0c).reshape(-1)
    c0 = (scale * (1.0 - fyp)).reshape(-1)
    c1 = (scale * fyp).reshape(-1)
    fxp = fxp.reshape(-1)

    pad = PXP - PX
    idx = np.concatenate([idx, np.zeros(pad, np.int64)])
    fxp = np.concatenate([fxp, np.zeros(pad, np.float32)])
    c0 = np.concatenate([c0, np.zeros(pad, np.float32)])
    c1 = np.concatenate([c1, np.zeros(pad, np.float32)])
    return idx, fxp, c0, c1


def _host_warp_mask(mask_bj, theta, h0):
    """Bilinear warp of one [H,W] mask (zero padding) for rows [h0,h0+R)."""
    ys = np.linspace(-1.0, 1.0, H, dtype=np.float32)[h0 : h0 + R]
    xs = np.linspace(-1.0, 1.0, W, dtype=np.float32)
    gx, gy = np.meshgrid(xs, ys)
    sx = theta[0, 0] * gx + theta[0, 1] * gy + theta[0, 2]
    sy = theta[1, 0] * gx + theta[1, 1] * gy + theta[1, 2]
    px = (sx + 1.0) * (W - 1) / 2.0
    py = (sy + 1.0) * (H - 1) / 2.0
    x0 = np.floor(px).astype(np.int64)
    y0 = np.floor(py).astype(np.int64)
    wx = (px - x0).astype(np.float32)
    wy = (py - y0).astype(np.float32)

    def gat(xi, yi):
        inb = ((xi >= 0) & (xi < W) & (yi >= 0) & (yi < H)).astype(np.float32)
        v = mask_bj[np.clip(yi, 0, H - 1), np.clip(xi, 0, W - 1)]
        return v * inb

    out = (
        gat(x0, y0) * (1 - wx) * (1 - wy)
        + gat(x0 + 1, y0) * wx * (1 - wy)
        + gat(x0, y0 + 1) * (1 - wx) * wy
        + gat(x0 + 1, y0 + 1) * wx * wy
    )
    return out.reshape(-1)  # [PX]


def _scene_layout(scene_of):
    """start/count per scene (agents are grouped by scene, in order)."""
    nb = max(scene_of) + 1
    start = [None] * nb
    cnt = [0] * nb
    for a, b in enumerate(scene_of):
        if start[b] is None:
            start[b] = a
        cnt[b] += 1
    return start, cnt


def _build_program(nagents, scene_of, src_names):
    """Build the SPMD Bass program (identical for all cores)."""
    nc = bacc.Bacc("TRN2", target_bir_lowering=False, num_devices=NCORES)
    NA = nagents
    sstart, scnt = _scene_layout(scene_of)

    srcs = [
        nc.dram_tensor(nm, [NENT + 1, 2 * C], F16, kind="ExternalInput")
        for nm in src_names
    ]
    idx_all = nc.dram_tensor("idx_all", [128, nagents * (PXP // 16)], I16,
                             kind="ExternalInput")
    scal_all = nc.dram_tensor("scal_all", [128, nagents * 8 * NT], F16,
                              kind="ExternalInput")
    ego_all = nc.dram_tensor("ego_all", [C, B * PXP], F16, kind="ExternalInput")
    cmb = nc.dram_tensor("cmb", [128, NT * 2 * NA], F16, kind="ExternalInput")
    w1 = nc.dram_tensor("w1", [2 * C, 2 * C], F16, kind="ExternalInput")
    w2 = nc.dram_tensor("w2", [2 * C, 32], F16, kind="ExternalInput")
    w3 = nc.dram_tensor("w3", [32, 32], F16, kind="ExternalInput")
    w4 = nc.dram_tensor("w4", [8, 32], F16, kind="ExternalInput")
    mlpw65 = nc.dram_tensor("mlpw65", [C + 1, C], F16, kind="ExternalInput")
    # per-partition scale/bias vectors: [128, 6] f32
    sb = nc.dram_tensor("sb", [128, 6], F32, kind="ExternalInput")
    cb4b = nc.dram_tensor("cb4b", [65, 1], F32, kind="ExternalInput")
    sb2 = nc.dram_tensor("sb2", [96, 1], F32, kind="ExternalInput")
    sb3 = nc.dram_tensor("sb3", [72, 1], F32, kind="ExternalInput")
    ident = nc.dram_tensor("ident", [128, 128], F16, kind="ExternalInput")
    id9 = nc.dram_tensor("id9", [NA, 16], F16, kind="ExternalInput")
    out = nc.dram_tensor("out", [B * C, PX], F32, kind="ExternalOutput")

    with TileContext(nc) as tc:
        with (
            tc.tile_pool(name="const", bufs=1) as cpool,
            tc.tile_pool(name="zs", bufs=1) as zpool,
            tc.tile_pool(name="work", bufs=2) as wpool,
            tc.tile_pool(name="att", bufs=1) as apool,
            tc.tile_pool(name="pmm", bufs=1, space="PSUM") as pmm,
            tc.tile_pool(name="ptr", bufs=2, space="PSUM") as ptr,
        ):
            # ---- constants ----
            idx_t = cpool.tile([128, nagents * (PXP // 16)], I16)
            nc.sync.dma_start(idx_t[:], idx_all[:, :])
            scal_t = cpool.tile([128, nagents * 8 * NT], F16)
            nc.sync.dma_start(scal_t[:], scal_all[:, :])
            ego_t = cpool.tile([C, B * PXP], F16)
            nc.sync.dma_start(ego_t[:], ego_all[:, :])
            cmb_t = cpool.tile([128, NT, 2 * NA], F16)
            nc.sync.dma_start(cmb_t[:], cmb[:, :].rearrange(
                "p (t a) -> p t a", a=2 * NA))
            w1_t = cpool.tile([2 * C, 2 * C], F16)
            nc.sync.dma_start(w1_t[:], w1[:, :])
            w2_t = cpool.tile([2 * C, 32], F16)
            nc.sync.dma_start(w2_t[:], w2[:, :])
            w3_t = cpool.tile([128, 32], F16)
            w4_t = cpool.tile([128, 32], F16)
            for q in range(3):
                nc.sync.dma_start(w3_t[32 * q : 32 * q + 32, :], w3[:, :])
                nc.sync.dma_start(w4_t[32 * q : 32 * q + 8, :], w4[:, :])
            mlpw_t = cpool.tile([C + 1, C], F16)
            nc.sync.dma_start(mlpw_t[:], mlpw65[:, :])
            sb_t = cpool.tile([128, 6], F32)
            nc.sync.dma_start(sb_t[:], sb[:, :])
            cb4_t = cpool.tile([65, 1], F32)
            nc.sync.dma_start(cb4_t[:], cb4b[:, :])
            id_t = cpool.tile([128, 128], F16)
            nc.sync.dma_start(id_t[:], ident[:, :])
            id9_t = cpool.tile([NA, 16], F16)
            nc.sync.dma_start(id9_t[:], id9[:, :])

            z_all = [zpool.tile([128, PXP], F16, name=f"z{j}", tag=f"z{j}")
                     for j in range(nagents)]
            nbr_all = [zpool.tile([128, NT, C], F16, name=f"nbr{j}",
                                  tag=f"nbr{j}")
                       for j in range(nagents)]
            h1_all = [None, None, None]
            sb2_t = cpool.tile([96, 1], F32)
            nc.sync.dma_start(sb2_t[:], sb2[:, :])
            sb3_t = cpool.tile([72, 1], F32)
            nc.sync.dma_start(sb3_t[:], sb3[:, :])
            s_all = apool.tile([nagents, PXP], F16)

            for j in range(nagents):
                b = scene_of[j]
                # ---- gather 2x2 taps, pixel-major ----
                g_t = wpool.tile([128, NT, 4 * C], F16, tag="g", bufs=3)
                src_flat = srcs[j][:, :].rearrange("a b -> (a b)")
                src_win = dataclasses.replace(
                    src_flat, ap=[[2 * C, NENT], [1, 4 * C]]
                )
                for (gt0, gtn) in ((0, 7), (7, 6)):
                    nc.gpsimd.dma_gather(
                        g_t[:, gt0 : gt0 + gtn, :],
                        src_win,
                        idx_t[:, j * (PXP // 16) + gt0 * 8 :
                              j * (PXP // 16) + (gt0 + gtn) * 8],
                        num_idxs=gtn * 128,
                        num_idxs_reg=gtn * 128,
                        elem_size=4 * C,
                        elem_step=2 * C,
                        single_packet=False,
                    )
                # ---- bilinear combine: nbr = w00*A+w01*B + w10*C+w11*D ----
                t1_t = wpool.tile([128, NT, 2 * C], F16, tag="t1", bufs=3)
                t2_t = wpool.tile([128, NT, 2 * C], F16, tag="t2", bufs=3)
                nbr_t = nbr_all[j]
                wq = scal_t[:, j * 8 * NT : (j + 1) * 8 * NT]
                for q, dst in ((0, t1_t[:, :, 0:C]), (1, t1_t[:, :, C : 2 * C]),
                               (2, t2_t[:, :, 0:C]), (3, t2_t[:, :, C : 2 * C])):
                    w_ap = dataclasses.replace(
                        wq, offset=wq.offset + 2 * q,
                        ap=[list(wq.ap[0]), [8, NT], [0, C // 2], [1, 2]])
                    src = g_t[:, :, q * C : (q + 1) * C]
                    nc.vector.tensor_tensor(
                        dst.rearrange("p a (c d) -> p a c d", d=2),
                        src.rearrange("p a (c d) -> p a c d", d=2),
                        w_ap, Alu.mult)
                nc.vector.tensor_tensor(t1_t[:, :, 0:C], t1_t[:, :, 0:C],
                                        t2_t[:, :, 0:C], Alu.add)
                nc.vector.tensor_tensor(t1_t[:, :, C : 2 * C],
                                        t1_t[:, :, C : 2 * C],
                                        t2_t[:, :, C : 2 * C], Alu.add)
                nc.vector.tensor_tensor(
                    nbr_t[:], t1_t[:, :, 0:C], t1_t[:, :, C : 2 * C], Alu.add)
                # ---- transpose px-major -> channel-major into z ----
                z_t = z_all[j]
                for t0 in range(0, NT, 4):
                    tn = min(4, NT - t0)
                    tr_ps = ptr.tile([C, 512], F16, tag="tr")
                    for t in range(t0, t0 + tn):
                        nc.tensor.transpose(
                            tr_ps[:, 128 * (t - t0) : 128 * (t - t0 + 1)],
                            nbr_t[:, t, :], id_t[:])
                    dst = z_t[0:C, 128 * t0 : 128 * (t0 + tn)]
                    nc.scalar.activation(dst, tr_ps[:, 0 : 128 * tn],
                                         Act.Copy)
                # ego half
                nc.sync.dma_start(
                    z_t[C : 2 * C, :], ego_t[:, b * PXP : (b + 1) * PXP]
                )
                # ---- conv1 + h1 (per agent; trio stages run below) ----
                h1_j = wpool.tile([128, PXP], F16, name=f"h1_{j}", tag=f"h1_{j % 3}")
                h1_all[j % 3] = h1_j
                for (o, n, mms) in HCHUNKS:
                    p1 = pmm.tile([128, 832], F32, tag="p1", bufs=1)
                    for (mo, mn) in mms:
                        nc.tensor.matmul(p1[:, mo : mo + mn], w1_t[:],
                                         z_t[:, o + mo : o + mo + mn],
                                         start=True, stop=True)
                    nc.scalar.activation(h1_j[:, o : o + n], p1[:, 0:n],
                                         Act.Relu, bias=sb_t[:, 1:2], scale=1.0)

                # ---- conv2..4 for a completed trio of agents ----
                if j % 3 == 2 or j == nagents - 1:
                    trio = [jj for jj in (j - j % 3 + q for q in range(3))
                            if jj <= j]
                    hs2 = wpool.tile([96, PXP], F16, tag="hs2")
                    hs3 = wpool.tile([72, PXP], F16, tag="hs3")
                    srow = wpool.tile([65, PXP], F16, tag="srow")
                    for (o, n, mms) in HCHUNKS:
                        sl = slice(o, o + n)
                        ph2 = pmm.tile([96, 832], F32, tag="p34", bufs=2)
                        for q, jj in enumerate(trio):
                            for (mo, mn) in mms:
                                nc.tensor.matmul(
                                    ph2[32 * q : 32 * q + 32, mo : mo + mn],
                                    w2_t[:],
                                    h1_all[q][:, o + mo : o + mo + mn],
                                    start=True, stop=True)
                        nc.scalar.activation(hs2[0 : 32 * len(trio), sl],
                                             ph2[0 : 32 * len(trio), 0:n],
                                             Act.Relu,
                                             bias=sb2_t[0 : 32 * len(trio), 0:1],
                                             scale=1.0)
                        p34 = pmm.tile([96, 832], F32, tag="p34", bufs=2)
                        for q, jj in enumerate(trio):
                            for (mo, mn) in mms:
                                nc.tensor.matmul(
                                    p34[32 * q : 32 * q + 32, mo : mo + mn],
                                    w3_t[32 * q : 32 * q + 32, :],
                                    hs2[32 * q : 32 * q + 32,
                                        o + mo : o + mo + mn],
                                    start=True, stop=True)
                        nc.scalar.activation(
                            hs3[0 : 32 * (len(trio) - 1) + 8, sl],
                            p34[0 : 32 * (len(trio) - 1) + 8, 0:n], Act.Relu,
                            bias=sb3_t[0 : 32 * (len(trio) - 1) + 8, 0:1],
                            scale=1.0)
                        p4 = pmm.tile([96, 832], F32, tag="p34", bufs=2)
                        for q, jj in enumerate(trio):
                            for (mo, mn) in mms:
                                nc.tensor.matmul(
                                    p4[32 * q : 32 * q + 32, mo : mo + mn],
                                    w4_t[32 * q : 32 * q + 8, :],
                                    hs3[32 * q : 32 * q + 8,
                                        o + mo : o + mo + mn],
                                    start=True, stop=True)
                        nc.scalar.activation(srow[0 : 32 * (len(trio) - 1) + 1, sl],
                                             p4[0 : 32 * (len(trio) - 1) + 1, 0:n],
                                             Act.Relu,
                                             bias=cb4_t[0 : 32 * (len(trio) - 1) + 1, 0:1],
                                             scale=1.0)
                    for q, jj in enumerate(trio):
                        nc.sync.dma_start(s_all[jj : jj + 1, :],
                                          srow[32 * q : 32 * q + 1, :])

            # ---- attention in pixel-major ----
            # transpose s_all [NA, px] -> s_ps [128, NT, 16] (col = agent)
            s_ps = pmm.tile([128, NT, 16], F32, tag="p34", bufs=2)
            for t in range(NT):
                nc.tensor.matmul(s_ps[:, t, :],
                                 s_all[:, 128 * t : 128 * (t + 1)],
                                 id9_t[:], start=True, stop=True)
            # e = exp(s) (s is already relu'd + biased)
            e_t = apool.tile([128, NT, NA], F16)
            nc.scalar.activation(e_t[:], s_ps[:, :, 0:NA], Act.Exp)
            # ep = e * (cm != 0); al = e * cm
            ep_t = apool.tile([128, NT, NA], F16)
            nc.vector.tensor_tensor(ep_t[:], e_t[:], cmb_t[:, :, NA : 2 * NA],
                                    Alu.mult)
            al_t = apool.tile([128, NT, NA], F16)
            nc.vector.tensor_tensor(al_t[:], e_t[:], cmb_t[:, :, 0:NA],
                                    Alu.mult)
            # den per scene (chain adds over the scene's agent columns)
            den_t = apool.tile([128, NT, B], F16)
            for b in range(B):
                cols = list(range(sstart[b], sstart[b] + scnt[b]))
                nc.vector.tensor_tensor(
                    den_t[:, :, b : b + 1], ep_t[:, :, cols[0] : cols[0] + 1],
                    ep_t[:, :, cols[1] : cols[1] + 1], Alu.add)
                for ck in cols[2:]:
                    nc.vector.tensor_tensor(
                        den_t[:, :, b : b + 1], den_t[:, :, b : b + 1],
                        ep_t[:, :, ck : ck + 1], Alu.add)
            rec_t = apool.tile([128, NT, B], F16)
            with nc.allow_low_precision(reason="den>=1, fp16 rec ok"):
                nc.vector.reciprocal(rec_t[:], den_t[:])
            # alpha = al * rec[scene]
            alp_t = apool.tile([128, NT, NA], F16)
            for b in range(B):
                s0, nj = sstart[b], scnt[b]
                rsl = rec_t[:, :, b : b + 1]
                r_ap = dataclasses.replace(
                    rsl, ap=[list(rsl.ap[0]), [B, NT], [0, nj]])
                nc.vector.tensor_tensor(alp_t[:, :, s0 : s0 + nj],
                                        al_t[:, :, s0 : s0 + nj], r_ap,
                                        Alu.mult)
            # scaled_j = alpha_j * nbr_j  (pixel-major), fold per scene
            u_pm = [apool.tile([128, NT, C], F16, name=f"upm{b}")
                    for b in range(B)]
            scl_t = [apool.tile([128, NT, C], F16, name=f"scl{j}")
                     for j in range(nagents)]
            for j in range(nagents):
                asl = alp_t[:, :, j : j + 1]
                a_ap = dataclasses.replace(
                    asl, ap=[list(asl.ap[0]), [NA, NT], [0, C]])
                nc.vector.tensor_tensor(scl_t[j][:], nbr_all[j][:], a_ap,
                                        Alu.mult)
            for b in range(B):
                cols = list(range(sstart[b], sstart[b] + scnt[b]))
                nc.vector.tensor_tensor(u_pm[b][:], scl_t[cols[0]][:],
                                        scl_t[cols[1]][:], Alu.add)
                for ck in cols[2:]:
                    nc.vector.tensor_tensor(u_pm[b][:], u_pm[b][:],
                                            scl_t[ck][:], Alu.add)
            # ---- transpose u back to channel-major, MLP, write out ----
            for b in range(B):
                u_sb = apool.tile([C + 1, PXP], F16, name=f"usb{b}")
                nc.gpsimd.memset(u_sb[C : C + 1, :], 1.0)
                for (o, n) in UCHUNKS:
                    u_ps = ptr.tile([C, 512], F16, tag="tr")
                    for t in range(o // 128, (o + n) // 128):
                        nc.tensor.transpose(
                            u_ps[:, 128 * t - o : 128 * (t + 1) - o],
                            u_pm[b][:, t, :], id_t[:])
                    nc.scalar.activation(u_sb[0:C, o : o + n], u_ps[:, 0:n],
                                         Act.Copy)
                for (o, n, mms) in MCHUNKS:
                    mps = pmm.tile([C, 832], F32, tag="p34", bufs=2)
                    for (mo, mn) in mms:
                        nc.tensor.matmul(mps[:, mo : mo + mn], mlpw_t[:],
                                         u_sb[:, o + mo : o + mo + mn],
                                         start=True, stop=True)
                    ob = wpool.tile([C, 832], F32, tag="ob")
                    nc.scalar.activation(ob[:, 0:n], mps[:, 0:n], Act.Copy)
                    nc.sync.dma_start(out[b * C : (b + 1) * C, o : o + n],
                                      ob[:, 0:n])

    nc.compile()
    return nc


_PROG_CACHE = {}
_LAST_RES = None


def kernel(**inputs):
    x = np.asarray(inputs["x"], np.float32)
    mask = np.asarray(inputs["mask"], np.float32)
    record_len = np.asarray(inputs["record_len"])
    ptm = np.asarray(inputs["pairwise_t_matrix"], np.float32)
    rec = [int(v) for v in record_len]
    agents = [(b, j) for b in range(B) for j in range(rec[b])]
    nagents = len(agents)
    scene_of = [b for (b, j) in agents]
    NA = nagents

    # ---- regroup x into per-scene node features ----
    node = np.zeros((B, L, C, H, W), np.float32)
    idx0 = 0
    for b, n in enumerate(rec):
        node[b, :n] = x[idx0 : idx0 + n]
        idx0 += n

    # ---- gather sources: dup-row pixel-major fp16 ----
    src_names = [f"src{a}" for a in range(nagents)]
    src_arrs = {}
    for a, (b, j) in enumerate(agents):
        feat = node[b, j]  # [C, H, W]
        ent = np.zeros((H + 1, W, 2 * C), np.float16)
        pm = feat.transpose(1, 2, 0).astype(np.float16)  # [H, W, C]
        ent[:H, :, :C] = pm
        ent[:H - 1, :, C:] = pm[1:]
        arr = np.zeros((NENT + 1, 2 * C), np.float16)
        arr[:NENT] = ent[:H].reshape(NENT, 2 * C)
        src_arrs[src_names[a]] = arr

    # ---- per-core index/scalar/mask/ego prep ----
    per_core = []
    for k in range(NCORES):
        h0 = k * R
        idx_cols = np.zeros((128, nagents * (PXP // 16)), np.int16)
        scal_cols = np.zeros((128, nagents * 8 * NT), np.float16)
        cmb_arr = np.zeros((128, NT * 2 * NA), np.float16)
        ego_arr = np.zeros((C, B * PXP), np.float16)
        for b in range(B):
            ego = node[b, 0][:, h0 : h0 + R, :].reshape(C, PX)
            ego_arr[:, b * PXP : b * PXP + PX] = ego.astype(np.float16)
        for a, (b, j) in enumerate(agents):
            theta = ptm[b, j, 0]  # theta[b, i=0, j] = ptm[b, j, 0]
            idx, fxp, c0, c1 = _host_warp_prep(theta, h0)
            idx_cols[:, a * (PXP // 16) : (a + 1) * (PXP // 16)] = _wrap_idx(idx)
            w00 = (c0 * (1.0 - fxp)).astype(np.float16)
            w10 = (c1 * (1.0 - fxp)).astype(np.float16)
            w01 = (c0 * fxp).astype(np.float16)
            w11 = (c1 * fxp).astype(np.float16)
            sc = scal_cols[:, a * 8 * NT : (a + 1) * 8 * NT]
            for t in range(NT):
                pxs = slice(128 * t, 128 * (t + 1))
                for q, wv in enumerate((w00, w10, w01, w11)):
                    sc[:, 8 * t + 2 * q] = wv[pxs]
                    sc[:, 8 * t + 2 * q + 1] = wv[pxs]
            wm = _host_warp_mask(mask[b, j], theta, h0)
            wmp = np.zeros(PXP, np.float32)
            wmp[:PX] = wm
            wmz = (wmp != 0).astype(np.float32)
            wmz[PX:] = 1.0  # keep den >= 1 on padded pixels
            # pixel-major: px = 128*t + p  ->  cmb[p, t*2NA + a] etc.
            cm_pm = wmp.reshape(NT, 128).T.astype(np.float16)   # [128, NT]
            cmz_pm = wmz.reshape(NT, 128).T.astype(np.float16)
            for t in range(NT):
                cmb_arr[:, t * 2 * NA + a] = cm_pm[:, t]
                cmb_arr[:, t * 2 * NA + NA + a] = cmz_pm[:, t]
        per_core.append((idx_cols, scal_cols, cmb_arr, ego_arr))

    # ---- shared small tensors ----
    def gf(n):
        return np.asarray(inputs[n], np.float32)

    sb = np.zeros((128, 6), np.float32)
    sb2v = np.zeros((96, 1), np.float32)
    sb3v = np.zeros((72, 1), np.float32)
    a1 = gf("g1") / np.sqrt(gf("rv1") + EPS)
    sb[:, 1] = gf("be1") + (gf("cb1") - gf("rm1")) * a1
    a2 = gf("g2") / np.sqrt(gf("rv2") + EPS)
    b2f = gf("be2") + (gf("cb2") - gf("rm2")) * a2
    a3 = gf("g3") / np.sqrt(gf("rv3") + EPS)
    b3f = gf("be3") + (gf("cb3") - gf("rm3")) * a3
    for q in range(3):
        sb2v[32 * q : 32 * q + 32, 0] = b2f
        sb3v[32 * q : 32 * q + 8, 0] = b3f

    mlp65 = np.zeros((C + 1, C), np.float16)
    mlp65[:C] = gf("mlp_w").astype(np.float16)
    mlp65[C] = gf("mlp_b").astype(np.float16)
    id9a = np.zeros((NA, 16), np.float16)
    id9a[:, :NA] = np.eye(NA, dtype=np.float16)

    shared = {
        "w1": (gf("w1") * a1[None, :]).astype(np.float16),
        "w2": (gf("w2") * a2[None, :]).astype(np.float16),
        "w3": np.pad((gf("w3") * a3[None, :]).astype(np.float16),
                     ((0, 0), (0, 24))),
        "w4": np.pad(gf("w4").astype(np.float16), ((0, 0), (0, 31))),
        "mlpw65": mlp65,
        "sb": sb,
        "sb2": sb2v,
        "sb3": sb3v,
        "cb4b": np.full((65, 1), gf("cb4")[0], np.float32),
        "ident": np.eye(128, dtype=np.float16),
        "id9": id9a,
    }
    shared.update(src_arrs)

    key = (nagents, tuple(scene_of))
    if key not in _PROG_CACHE:
        _PROG_CACHE[key] = _build_program(nagents, scene_of, src_names)
    nc = _PROG_CACHE[key]

    in_maps = []
    for k in range(NCORES):
        idx_cols, scal_cols, cmb_arr, ego_arr = per_core[k]
        m = dict(shared)
        m["idx_all"] = idx_cols
        m["scal_all"] = scal_cols
        m["cmb"] = cmb_arr
        m["ego_all"] = ego_arr
        in_maps.append(m)

    trace = bool(os.environ.get("KERNEL_TRACE"))
    res = run_bass_kernel_spmd(nc, in_maps, core_ids=list(range(NCORES)),
                               trace=trace)
    global _LAST_RES
    _LAST_RES = res

    out = np.zeros((B, C, H, W), np.float32)
    for k in range(NCORES):
        o = res.results[k]["out"]  # [B*C, PX]
        out[:, :, k * R : (k + 1) * R, :] = o.reshape(B, C, R, W)
    return out


# revision 10
# speedup vs baseline: 1.2856x; 1.1492x over previous
"""DiscoNetFusion Trainium2 kernel (8 NeuronCores, SPMD).

Strategy
--------
Only ego agent i=0 of each scene contributes to the output, so per scene b we
need the L_b = record_len[b] neighbor warps nbr[b,0,j], the 4-layer 1x1-conv
attention head on z=[nbr;ego], a softmax over j, and the weighted feature sum
followed by a channel MLP.

Core k handles output rows [10k, 10k+10) of ALL scenes (8 cores x 10 rows =
80 rows).  Per core there are sum(record_len)=9 (scene, agent) units; each
unit is 1600 output pixels (padded to 1664 = 13 tiles of 128).

Bilinear warp = DMA gather (pixel-major dup-row fp16 source; one index
fetches the 2x2 tap patch) + a lerp combine on the vector engine with
per-partition (=per-pixel) scalars, then a PE transpose back to channel-major
for the conv stack.

Attention runs in PIXEL-major: s [9, px] is transposed by tiny PE matmuls to
[px-partition, tile, agent], so softmax/normalize ops shrink from
[9, 1664]-sized (927ns) to [128, 13, 9]-sized (~100-500ns) DVE ops.  The
weighted sum alpha*nbr reuses the pre-transpose pixel-major nbr tiles, gets
folded per scene with a few adds, and is transposed back to channel-major by
PE for the MLP.  The MLP bias rides as a 65th weight row against a constant
ones row in the moving operand, and the result is DMA'd to HBM directly from
PSUM.
"""

import dataclasses
import os

import numpy as np

import concourse.bacc as bacc
import concourse.mybir as mybir
from concourse.bass_utils import run_bass_kernel_spmd
from concourse.tile import TileContext

F32 = mybir.dt.float32
F16 = mybir.dt.float16
I16 = mybir.dt.int16
Alu = mybir.AluOpType
Act = mybir.ActivationFunctionType

C = 64
H = 80
W = 160
B = 3
L = 4
EPS = 1e-5
NCORES = 8
R = H // NCORES            # output rows per core
PX = R * W                 # 1600 real pixels
NT = 13                    # px tiles of 128
PXP = NT * 128             # 1664 padded pixels
NENT = H * W               # gather source entries per agent
HCHUNKS = [(0, 832, [(0, 512), (512, 320)]), (832, 832, [(0, 512), (512, 320)])]
# u transposes write 128-wide blocks; chunks must be tile-aligned
UCHUNKS = [(0, 512), (512, 512), (1024, 512), (1536, 128)]
MCHUNKS = [(0, 832, [(0, 512), (512, 320)]), (832, 768, [(0, 512), (512, 256)])]


def _wrap_idx(idx_flat):
    """[N] -> [128, N//16] wrapped-in-16-partitions, replicated to 8 groups."""
    n = idx_flat.shape[0]
    w = idx_flat.reshape(n // 16, 16).T  # [16, N//16]
    return np.tile(w, (8, 1)).astype(np.int16)


def _host_warp_prep(theta, h0):
    """Per-(unit) gather indices + lerp scalars for output rows [h0,h0+R).

    Returns idx[PXP] int32 (entry units), fx, c0, c1 [PXP] f32.
    """
    ys = np.linspace(-1.0, 1.0, H, dtype=np.float32)[h0 : h0 + R]
    xs = np.linspace(-1.0, 1.0, W, dtype=np.float32)
    gx, gy = np.meshgrid(xs, ys)  # [R, W]
    sx = theta[0, 0] * gx + theta[0, 1] * gy + theta[0, 2]
    sy = theta[1, 0] * gx + theta[1, 1] * gy + theta[1, 2]
    px = (sx + 1.0) * (W - 1) / 2.0
    py = (sy + 1.0) * (H - 1) / 2.0
    x0 = np.floor(px).astype(np.int64)
    y0 = np.floor(py).astype(np.int64)
    fx = (px - x0).astype(np.float32)
    fy = (py - y0).astype(np.float32)

    scale = np.ones_like(fx)
    # x handling
    x0c = np.clip(x0, 0, W - 1)
    fxp = fx.copy()
    m = x0 == W - 1          # x1 out of bounds -> drop B/D taps
    fxp[m] = 0.0
    scale[m] *= 1.0 - fx[m]
    m = x0 == -1             # x0 out of bounds -> entry at x=0 is the B tap
    x0c[m] = 0
    fxp[m] = 0.0
    scale[m] *= fx[m]
    m = (x0 < -1) | (x0 > W - 1)
    x0c[m] = 0
    fxp[m] = 0.0
    scale[m] = 0.0
    # y handling (entry [y0] holds rows y0,y0+1; row 80 half is zeros)
    y0c = np.clip(y0, 0, H - 1)
    fyp = fy.copy()
    m = y0 == -1             # row0 is the F tap
    y0c[m] = 0
    fyp[m] = 0.0
    scale[m] *= fy[m]
    m = (y0 < -1) | (y0 > H - 1)
    y0c[m] = 0
    fyp[m] = 0.0
    scale[m] = 0.0

    idx = (y0c * W + x0c).reshape(-1)
    c0 = (scale * (1.0 - fyp)).reshape(-1)
    c1 = (scale * fyp).reshape(-1)
    fxp = fxp.reshape(-1)

    pad = PXP - PX
    idx = np.concatenate([idx, np.zeros(pad, np.int64)])
    fxp = np.concatenate([fxp, np.zeros(pad, np.float32)])
    c0 = np.concatenate([c0, np.zeros(pad, np.float32)])
    c1 = np.concatenate([c1, np.zeros(pad, np.float32)])
    return idx, fxp, c0, c1


def _host_warp_mask(mask_bj, theta, h0):
    """Bilinear warp of one [H,W] mask (zero padding) for rows [h0,h0+R)."""
    ys = np.linspace(-1.0, 1.0, H, dtype=np.float32)[h0 : h0 + R]
    xs = np.linspace(-1.0, 1.0, W, dtype=np.float32)
    gx, gy = np.meshgrid(xs, ys)
    sx = theta[0, 0] * gx + theta[0, 1] * gy + theta[0, 2]
    sy = theta[1, 0] * gx + theta[1, 1] * gy + theta[1, 2]
    px = (sx + 1.0) * (W - 1) / 2.0
    py = (sy + 1.0) * (H - 1) / 2.0
    x0 = np.floor(px).astype(np.int64)
    y0 = np.floor(py).astype(np.int64)
    wx = (px - x0).astype(np.float32)
    wy = (py - y0).astype(np.float32)

    def gat(xi, yi):
        inb = ((xi >= 0) & (xi < W) & (yi >= 0) & (yi < H)).astype(np.float32)
        v = mask_bj[np.clip(yi, 0, H - 1), np.clip(xi, 0, W - 1)]
        return v * inb

    out = (
        gat(x0, y0) * (1 - wx) * (1 - wy)
        + gat(x0 + 1, y0) * wx * (1 - wy)
        + gat(x0, y0 + 1) * (1 - wx) * wy
        + gat(x0 + 1, y0 + 1) * wx * wy
    )
    return out.reshape(-1)  # [PX]


def _scene_layout(scene_of):
    """start/count per scene (agents are grouped by scene, in order)."""
    nb = max(scene_of) + 1
    start = [None] * nb
    cnt = [0] * nb
    for a, b in enumerate(scene_of):
        if start[b] is None:
            start[b] = a
        cnt[b] += 1
    return start, cnt


def _build_program(nagents, scene_of, src_names):
    """Build the SPMD Bass program (identical for all cores)."""
    nc = bacc.Bacc("TRN2", target_bir_lowering=False, num_devices=NCORES)
    NA = nagents
    sstart, scnt = _scene_layout(scene_of)

    srcs = [
        nc.dram_tensor(nm, [NENT + 1, 2 * C], F16, kind="ExternalInput")
        for nm in src_names
    ]
    idx_all = nc.dram_tensor("idx_all", [128, nagents * (PXP // 16)], I16,
                             kind="ExternalInput")
    scal_all = nc.dram_tensor("scal_all", [128, nagents * 8 * NT], F16,
                              kind="ExternalInput")
    ego_all = nc.dram_tensor("ego_all", [C, B * PXP], F16, kind="ExternalInput")
    cmb = nc.dram_tensor("cmb", [128, NT * 2 * NA], F16, kind="ExternalInput")
    w1 = nc.dram_tensor("w1", [2 * C, 2 * C], F16, kind="ExternalInput")
    w2 = nc.dram_tensor("w2", [2 * C, 32], F16, kind="ExternalInput")
    bd3 = nc.dram_tensor("bd3", [96, 96], F16, kind="ExternalInput")
    bd4 = nc.dram_tensor("bd4", [96, 3], F16, kind="ExternalInput")
    mlpw65 = nc.dram_tensor("mlpw65", [C + 1, C], F16, kind="ExternalInput")
    # per-partition scale/bias vectors: [128, 6] f32
    sb = nc.dram_tensor("sb", [128, 6], F32, kind="ExternalInput")
    cb4v = nc.dram_tensor("cb4v", [128, 1], F32, kind="ExternalInput")
    sb2 = nc.dram_tensor("sb2", [96, 1], F32, kind="ExternalInput")
    sb3 = nc.dram_tensor("sb3", [96, 1], F32, kind="ExternalInput")
    ident = nc.dram_tensor("ident", [128, 128], F16, kind="ExternalInput")
    out = nc.dram_tensor("out", [B * C, PX], F32, kind="ExternalOutput")

    with TileContext(nc) as tc:
        with (
            tc.tile_pool(name="const", bufs=1) as cpool,
            tc.tile_pool(name="zs", bufs=1) as zpool,
            tc.tile_pool(name="work", bufs=2) as wpool,
            tc.tile_pool(name="att", bufs=1) as apool,
            tc.tile_pool(name="pmm", bufs=1, space="PSUM") as pmm,
            tc.tile_pool(name="ptr", bufs=2, space="PSUM") as ptr,
        ):
            # ---- constants ----
            idx_t = cpool.tile([128, nagents * (PXP // 16)], I16)
            nc.sync.dma_start(idx_t[:], idx_all[:, :])
            scal_t = cpool.tile([128, nagents * 8 * NT], F16)
            nc.sync.dma_start(scal_t[:], scal_all[:, :])
            ego_t = cpool.tile([C, B * PXP], F16)
            nc.sync.dma_start(ego_t[:], ego_all[:, :])
            cmb_t = cpool.tile([128, NT, 2 * NA], F16)
            nc.sync.dma_start(cmb_t[:], cmb[:, :].rearrange(
                "p (t a) -> p t a", a=2 * NA))
            w1_t = cpool.tile([2 * C, 2 * C], F16)
            nc.sync.dma_start(w1_t[:], w1[:, :])
            w2_t = cpool.tile([2 * C, 32], F16)
            nc.sync.dma_start(w2_t[:], w2[:, :])
            bd3_t = cpool.tile([96, 96], F16)
            nc.sync.dma_start(bd3_t[:], bd3[:, :])
            bd4_t = cpool.tile([96, 3], F16)
            nc.sync.dma_start(bd4_t[:], bd4[:, :])
            mlpw_t = cpool.tile([C + 1, C], F16)
            nc.sync.dma_start(mlpw_t[:], mlpw65[:, :])
            sb_t = cpool.tile([128, 6], F32)
            nc.sync.dma_start(sb_t[:], sb[:, :])
            cb4_t = cpool.tile([128, 1], F32)
            nc.sync.dma_start(cb4_t[:], cb4v[:, :])
            id_t = cpool.tile([128, 128], F16)
            nc.sync.dma_start(id_t[:], ident[:, :])

            z_all = [zpool.tile([128, PXP], F16, name=f"z{j}", tag=f"z{j}")
                     for j in range(nagents)]
            nbr_all = [zpool.tile([128, NT, C], F16, name=f"nbr{j}",
                                  tag=f"nbr{j}")
                       for j in range(nagents)]
            h1_all = [None, None, None]
            sb2_t = cpool.tile([96, 1], F32)
            nc.sync.dma_start(sb2_t[:], sb2[:, :])
            sb3_t = cpool.tile([96, 1], F32)
            nc.sync.dma_start(sb3_t[:], sb3[:, :])
            # s (pixel-major) accumulates from the fused conv4+transpose mms
            s_ps = pmm.tile([128, NT, 16], F32, tag="s_ps", bufs=1)

            for j in range(nagents):
                b = scene_of[j]
                # ---- gather 2x2 taps, pixel-major ----
                g_t = wpool.tile([128, NT, 4 * C], F16, tag="g", bufs=3)
                src_flat = srcs[j][:, :].rearrange("a b -> (a b)")
                src_win = dataclasses.replace(
                    src_flat, ap=[[2 * C, NENT], [1, 4 * C]]
                )
                for (gt0, gtn) in ((0, 7), (7, 6)):
                    nc.gpsimd.dma_gather(
                        g_t[:, gt0 : gt0 + gtn, :],
                        src_win,
                        idx_t[:, j * (PXP // 16) + gt0 * 8 :
                              j * (PXP // 16) + (gt0 + gtn) * 8],
                        num_idxs=gtn * 128,
                        num_idxs_reg=gtn * 128,
                        elem_size=4 * C,
                        elem_step=2 * C,
                        single_packet=False,
                    )
                # ---- bilinear combine: nbr = w00*A+w01*B + w10*C+w11*D ----
                t1_t = wpool.tile([128, NT, 2 * C], F16, tag="t1", bufs=3)
                t2_t = wpool.tile([128, NT, 2 * C], F16, tag="t2", bufs=3)
                nbr_t = nbr_all[j]
                wq = scal_t[:, j * 8 * NT : (j + 1) * 8 * NT]
                for q, dst in ((0, t1_t[:, :, 0:C]), (1, t1_t[:, :, C : 2 * C]),
                               (2, t2_t[:, :, 0:C]), (3, t2_t[:, :, C : 2 * C])):
                    w_ap = dataclasses.replace(
                        wq, offset=wq.offset + 2 * q,
                        ap=[list(wq.ap[0]), [8, NT], [0, C // 2], [1, 2]])
                    src = g_t[:, :, q * C : (q + 1) * C]
                    nc.vector.tensor_tensor(
                        dst.rearrange("p a (c d) -> p a c d", d=2),
                        src.rearrange("p a (c d) -> p a c d", d=2),
                        w_ap, Alu.mult)
                nc.vector.tensor_tensor(t1_t[:, :, 0:C], t1_t[:, :, 0:C],
                                        t2_t[:, :, 0:C], Alu.add)
                nc.vector.tensor_tensor(t1_t[:, :, C : 2 * C],
                                        t1_t[:, :, C : 2 * C],
                                        t2_t[:, :, C : 2 * C], Alu.add)
                nc.vector.tensor_tensor(
                    nbr_t[:], t1_t[:, :, 0:C], t1_t[:, :, C : 2 * C], Alu.add)
                # ---- transpose px-major -> channel-major into z ----
                z_t = z_all[j]
                for t0 in range(0, NT, 4):
                    tn = min(4, NT - t0)
                    tr_ps = ptr.tile([C, 512], F16, tag="tr")
                    for t in range(t0, t0 + tn):
                        nc.tensor.transpose(
                            tr_ps[:, 128 * (t - t0) : 128 * (t - t0 + 1)],
                            nbr_t[:, t, :], id_t[:])
                    dst = z_t[0:C, 128 * t0 : 128 * (t0 + tn)]
                    nc.scalar.activation(dst, tr_ps[:, 0 : 128 * tn],
                                         Act.Copy)
                # ego half
                nc.sync.dma_start(
                    z_t[C : 2 * C, :], ego_t[:, b * PXP : (b + 1) * PXP]
                )
                # ---- conv1 + h1 (per agent; trio stages run below) ----
                h1_j = wpool.tile([128, PXP], F16, name=f"h1_{j}", tag=f"h1_{j % 3}")
                h1_all[j % 3] = h1_j
                for (o, n, mms) in HCHUNKS:
                    p1 = pmm.tile([128, 832], F32, tag="p34", bufs=2)
                    for (mo, mn) in mms:
                        nc.tensor.matmul(p1[:, mo : mo + mn], w1_t[:],
                                         z_t[:, o + mo : o + mo + mn],
                                         start=True, stop=True)
                    nc.scalar.activation(h1_j[:, o : o + n], p1[:, 0:n],
                                         Act.Relu, bias=sb_t[:, 1:2], scale=1.0)

                # ---- conv2..4 for a completed trio of agents ----
                if j % 3 == 2 or j == nagents - 1:
                    g = j // 3
                    trio = [jj for jj in (j - j % 3 + q for q in range(3))
                            if jj <= j]
                    nt_ = len(trio)
                    hs2 = wpool.tile([96, PXP], F16, tag="hs2")
                    hs3 = wpool.tile([96, PXP], F16, tag="hs3")
                    for (o, n, mms) in HCHUNKS:
                        sl = slice(o, o + n)
                        ph2 = pmm.tile([96, 832], F32, tag="p34", bufs=2)
                        for q, jj in enumerate(trio):
                            for (mo, mn) in mms:
                                nc.tensor.matmul(
                                    ph2[32 * q : 32 * q + 32, mo : mo + mn],
                                    w2_t[:],
                                    h1_all[q][:, o + mo : o + mo + mn],
                                    start=True, stop=True)
                        nc.scalar.activation(hs2[0 : 32 * nt_, sl],
                                             ph2[0 : 32 * nt_, 0:n],
                                             Act.Relu,
                                             bias=sb2_t[0 : 32 * nt_, 0:1],
                                             scale=1.0)
                        # conv3 via block-diag stationary: one mm per piece
                        p34 = pmm.tile([96, 832], F32, tag="p34", bufs=2)
                        for (mo, mn) in mms:
                            nc.tensor.matmul(
                                p34[0 : 32 * nt_, mo : mo + mn],
                                bd3_t[0 : 32 * nt_, 0 : 32 * nt_],
                                hs2[0 : 32 * nt_, o + mo : o + mo + mn],
                                start=True, stop=True)
                        nc.scalar.activation(
                            hs3[0 : 32 * nt_, sl],
                            p34[0 : 32 * nt_, 0:n], Act.Relu,
                            bias=sb3_t[0 : 32 * nt_, 0:1],
                            scale=1.0)
                    # conv4 fused with the s transpose: per px tile,
                    # s_pm[px, 3g+q] = sum_c w4[c] * h3_q[32q+c, px]
                    for t in range(NT):
                        nc.tensor.matmul(
                            s_ps[:, t, 3 * g : 3 * g + nt_],
                            hs3[0 : 32 * nt_, 128 * t : 128 * (t + 1)],
                            bd4_t[0 : 32 * nt_, 0:nt_],
                            start=True, stop=True)

            # ---- attention in pixel-major ----
            # e = exp(relu(s_raw + cb4)) = max(exp(s_raw + cb4), 1)
            e_t = apool.tile([128, NT, NA], F16)
            nc.scalar.activation(e_t[:], s_ps[:, :, 0:NA], Act.Exp,
                                 bias=cb4_t[:, 0:1], scale=1.0)
            nc.vector.tensor_scalar_max(e_t[:], e_t[:], 1.0)
            # ep = e * (cm != 0); al = e * cm
            ep_t = apool.tile([128, NT, NA], F16)
            nc.vector.tensor_tensor(ep_t[:], e_t[:], cmb_t[:, :, NA : 2 * NA],
                                    Alu.mult)
            al_t = apool.tile([128, NT, NA], F16)
            nc.vector.tensor_tensor(al_t[:], e_t[:], cmb_t[:, :, 0:NA],
                                    Alu.mult)
            # den per scene (chain adds over the scene's agent columns)
            den_t = apool.tile([128, NT, B], F16)
            for b in range(B):
                cols = list(range(sstart[b], sstart[b] + scnt[b]))
                nc.vector.tensor_tensor(
                    den_t[:, :, b : b + 1], ep_t[:, :, cols[0] : cols[0] + 1],
                    ep_t[:, :, cols[1] : cols[1] + 1], Alu.add)
                for ck in cols[2:]:
                    nc.vector.tensor_tensor(
                        den_t[:, :, b : b + 1], den_t[:, :, b : b + 1],
                        ep_t[:, :, ck : ck + 1], Alu.add)
            rec_t = apool.tile([128, NT, B], F16)
            with nc.allow_low_precision(reason="den>=1, fp16 rec ok"):
                nc.vector.reciprocal(rec_t[:], den_t[:])
            # alpha = al * rec[scene]
            alp_t = apool.tile([128, NT, NA], F16)
            for b in range(B):
                s0, nj = sstart[b], scnt[b]
                rsl = rec_t[:, :, b : b + 1]
                r_ap = dataclasses.replace(
                    rsl, ap=[list(rsl.ap[0]), [B, NT], [0, nj]])
                nc.vector.tensor_tensor(alp_t[:, :, s0 : s0 + nj],
                                        al_t[:, :, s0 : s0 + nj], r_ap,
                                        Alu.mult)
            # scaled_j = alpha_j * nbr_j  (pixel-major), fold per scene
            u_pm = [apool.tile([128, NT, C], F16, name=f"upm{b}")
                    for b in range(B)]
            scl_t = [apool.tile([128, NT, C], F16, name=f"scl{j}")
                     for j in range(nagents)]
            for j in range(nagents):
                asl = alp_t[:, :, j : j + 1]
                a_ap = dataclasses.replace(
                    asl, ap=[list(asl.ap[0]), [NA, NT], [0, C]])
                nc.vector.tensor_tensor(scl_t[j][:], nbr_all[j][:], a_ap,
                                        Alu.mult)
            for b in range(B):
                cols = list(range(sstart[b], sstart[b] + scnt[b]))
                nc.vector.tensor_tensor(u_pm[b][:], scl_t[cols[0]][:],
                                        scl_t[cols[1]][:], Alu.add)
                for ck in cols[2:]:
                    nc.vector.tensor_tensor(u_pm[b][:], u_pm[b][:],
                                            scl_t[ck][:], Alu.add)
            # ---- transpose u back to channel-major, MLP, write out ----
            for b in range(B):
                u_sb = apool.tile([C + 1, PXP], F16, name=f"usb{b}")
                nc.gpsimd.memset(u_sb[C : C + 1, :], 1.0)
                for (o, n) in UCHUNKS:
                    u_ps = ptr.tile([C, 512], F16, tag="tr")
                    for t in range(o // 128, (o + n) // 128):
                        nc.tensor.transpose(
                            u_ps[:, 128 * t - o : 128 * (t + 1) - o],
                            u_pm[b][:, t, :], id_t[:])
                    nc.scalar.activation(u_sb[0:C, o : o + n], u_ps[:, 0:n],
                                         Act.Copy)
                for (o, n, mms) in MCHUNKS:
                    mps = pmm.tile([C, 832], F32, tag="p34", bufs=2)
                    for (mo, mn) in mms:
                        nc.tensor.matmul(mps[:, mo : mo + mn], mlpw_t[:],
                                         u_sb[:, o + mo : o + mo + mn],
                                         start=True, stop=True)
                    ob = wpool.tile([C, 832], F32, tag="ob")
                    nc.scalar.activation(ob[:, 0:n], mps[:, 0:n], Act.Copy)
                    nc.sync.dma_start(out[b * C : (b + 1) * C, o : o + n],
                                      ob[:, 0:n])

    nc.compile()
    return nc


_PROG_CACHE = {}
_LAST_RES = None


def kernel(**inputs):
    x = np.asarray(inputs["x"], np.float32)
    mask = np.asarray(inputs["mask"], np.float32)
    record_len = np.asarray(inputs["record_len"])
    ptm = np.asarray(inputs["pairwise_t_matrix"], np.float32)
    rec = [int(v) for v in record_len]
    agents = [(b, j) for b in range(B) for j in range(rec[b])]
    nagents = len(agents)
    scene_of = [b for (b, j) in agents]
    NA = nagents

    # ---- regroup x into per-scene node features ----
    node = np.zeros((B, L, C, H, W), np.float32)
    idx0 = 0
    for b, n in enumerate(rec):
        node[b, :n] = x[idx0 : idx0 + n]
        idx0 += n

    # ---- gather sources: dup-row pixel-major fp16 ----
    src_names = [f"src{a}" for a in range(nagents)]
    src_arrs = {}
    for a, (b, j) in enumerate(agents):
        feat = node[b, j]  # [C, H, W]
        ent = np.zeros((H + 1, W, 2 * C), np.float16)
        pm = feat.transpose(1, 2, 0).astype(np.float16)  # [H, W, C]
        ent[:H, :, :C] = pm
        ent[:H - 1, :, C:] = pm[1:]
        arr = np.zeros((NENT + 1, 2 * C), np.float16)
        arr[:NENT] = ent[:H].reshape(NENT, 2 * C)
        src_arrs[src_names[a]] = arr

    # ---- per-core index/scalar/mask/ego prep ----
    per_core = []
    for k in range(NCORES):
        h0 = k * R
        idx_cols = np.zeros((128, nagents * (PXP // 16)), np.int16)
        scal_cols = np.zeros((128, nagents * 8 * NT), np.float16)
        cmb_arr = np.zeros((128, NT * 2 * NA), np.float16)
        ego_arr = np.zeros((C, B * PXP), np.float16)
        for b in range(B):
            ego = node[b, 0][:, h0 : h0 + R, :].reshape(C, PX)
            ego_arr[:, b * PXP : b * PXP + PX] = ego.astype(np.float16)
        for a, (b, j) in enumerate(agents):
            theta = ptm[b, j, 0]  # theta[b, i=0, j] = ptm[b, j, 0]
            idx, fxp, c0, c1 = _host_warp_prep(theta, h0)
            idx_cols[:, a * (PXP // 16) : (a + 1) * (PXP // 16)] = _wrap_idx(idx)
            w00 = (c0 * (1.0 - fxp)).astype(np.float16)
            w10 = (c1 * (1.0 - fxp)).astype(np.float16)
            w01 = (c0 * fxp).astype(np.float16)
            w11 = (c1 * fxp).astype(np.float16)
            sc = scal_cols[:, a * 8 * NT : (a + 1) * 8 * NT]
            for t in range(NT):
                pxs = slice(128 * t, 128 * (t + 1))
                for q, wv in enumerate((w00, w10, w01, w11)):
                    sc[:, 8 * t + 2 * q] = wv[pxs]
                    sc[:, 8 * t + 2 * q + 1] = wv[pxs]
            wm = _host_warp_mask(mask[b, j], theta, h0)
            wmp = np.zeros(PXP, np.float32)
            wmp[:PX] = wm
            wmz = (wmp != 0).astype(np.float32)
            wmz[PX:] = 1.0  # keep den >= 1 on padded pixels
            # pixel-major: px = 128*t + p  ->  cmb[p, t*2NA + a] etc.
            cm_pm = wmp.reshape(NT, 128).T.astype(np.float16)   # [128, NT]
            cmz_pm = wmz.reshape(NT, 128).T.astype(np.float16)
            for t in range(NT):
                cmb_arr[:, t * 2 * NA + a] = cm_pm[:, t]
                cmb_arr[:, t * 2 * NA + NA + a] = cmz_pm[:, t]
        per_core.append((idx_cols, scal_cols, cmb_arr, ego_arr))

    # ---- shared small tensors ----
    def gf(n):
        return np.asarray(inputs[n], np.float32)

    sb = np.zeros((128, 6), np.float32)
    sb2v = np.zeros((96, 1), np.float32)
    sb3v = np.zeros((96, 1), np.float32)
    a1 = gf("g1") / np.sqrt(gf("rv1") + EPS)
    sb[:, 1] = gf("be1") + (gf("cb1") - gf("rm1")) * a1
    a2 = gf("g2") / np.sqrt(gf("rv2") + EPS)
    b2f = gf("be2") + (gf("cb2") - gf("rm2")) * a2
    a3 = gf("g3") / np.sqrt(gf("rv3") + EPS)
    b3f = gf("be3") + (gf("cb3") - gf("rm3")) * a3
    for q in range(3):
        sb2v[32 * q : 32 * q + 32, 0] = b2f
        sb3v[32 * q : 32 * q + 8, 0] = b3f

    w3f = (gf("w3") * a3[None, :]).astype(np.float16)  # [32, 8]
    w4f = gf("w4").astype(np.float16)                  # [8, 1]
    bd3a = np.zeros((96, 96), np.float16)
    bd4a = np.zeros((96, 3), np.float16)
    for q in range(3):
        bd3a[32 * q : 32 * q + 32, 32 * q : 32 * q + 8] = w3f
        bd4a[32 * q : 32 * q + 8, q] = w4f[:, 0]

    mlp65 = np.zeros((C + 1, C), np.float16)
    mlp65[:C] = gf("mlp_w").astype(np.float16)
    mlp65[C] = gf("mlp_b").astype(np.float16)

    shared = {
        "w1": (gf("w1") * a1[None, :]).astype(np.float16),
        "w2": (gf("w2") * a2[None, :]).astype(np.float16),
        "bd3": bd3a,
        "bd4": bd4a,
        "mlpw65": mlp65,
        "sb": sb,
        "sb2": sb2v,
        "sb3": sb3v,
        "cb4v": np.full((128, 1), gf("cb4")[0], np.float32),
        "ident": np.eye(128, dtype=np.float16),
    }
    shared.update(src_arrs)

    key = (nagents, tuple(scene_of))
    if key not in _PROG_CACHE:
        _PROG_CACHE[key] = _build_program(nagents, scene_of, src_names)
    nc = _PROG_CACHE[key]

    in_maps = []
    for k in range(NCORES):
        idx_cols, scal_cols, cmb_arr, ego_arr = per_core[k]
        m = dict(shared)
        m["idx_all"] = idx_cols
        m["scal_all"] = scal_cols
        m["cmb"] = cmb_arr
        m["ego_all"] = ego_arr
        in_maps.append(m)

    trace = bool(os.environ.get("KERNEL_TRACE"))
    res = run_bass_kernel_spmd(nc, in_maps, core_ids=list(range(NCORES)),
                               trace=trace)
    global _LAST_RES
    _LAST_RES = res

    out = np.zeros((B, C, H, W), np.float32)
    for k in range(NCORES):
        o = res.results[k]["out"]  # [B*C, PX]
        out[:, :, k * R : (k + 1) * R, :] = o.reshape(B, C, R, W)
    return out


# revision 21
# speedup vs baseline: 1.3872x; 1.0790x over previous
"""DiscoNetFusion Trainium2 kernel (8 NeuronCores, SPMD).

Strategy
--------
Only ego agent i=0 of each scene contributes to the output, so per scene b we
need the L_b = record_len[b] neighbor warps nbr[b,0,j], the 4-layer 1x1-conv
attention head on z=[nbr;ego], a softmax over j, and the weighted feature sum
followed by a channel MLP.

Core k handles output rows [10k, 10k+10) of ALL scenes (8 cores x 10 rows =
80 rows).  Per core there are sum(record_len)=9 (scene, agent) units; each
unit is 1600 output pixels (padded to 1664 = 13 tiles of 128).

Ego agents (j=0 of each scene) have an exact-identity warp, so the host
ships their features directly in both channel-major and pixel-major layout
and they skip the gather/lerp/transpose path entirely.

The remaining agents are processed in PAIRS sharing tiles: one DMA gather
per pair (concatenated dup-row source, indices offset by the source length),
one set of lerp ops covering both agents ([128, 26, C] pixel-major), one PE
transpose per px tile yielding both agents' channel-major rows at once.
conv1 is split into a nbr-half and an ego-half matmul accumulating in PSUM
(the z=[nbr;ego] concat never materializes; the ego half reuses the shared
per-scene channel-major ego tile).

conv3 uses a block-diagonal stationary (1 matmul per piece per group of 3
agents); conv4 is FUSED into the s-transpose: per px tile a tiny matmul with
the hs3 tile as stationary and a block-column w4 moving operand writes
s[px, col] directly in pixel-major PSUM.  Softmax + attention then run in
pixel-major where every op is [128, 13, 9]-sized (~100-500ns).  The weighted
sum reuses the pixel-major nbr tiles, folds per scene, and is transposed
back by PE; the MLP bias rides as a 65th weight row against a ones row.
"""

import dataclasses
import os

import numpy as np

import concourse.bacc as bacc
import concourse.mybir as mybir
from concourse.bass_utils import run_bass_kernel_spmd
from concourse.tile import TileContext

F32 = mybir.dt.float32
F16 = mybir.dt.float16
I16 = mybir.dt.int16
Alu = mybir.AluOpType
Act = mybir.ActivationFunctionType

C = 64
H = 80
W = 160
B = 3
L = 4
EPS = 1e-5
NCORES = 8
R = H // NCORES            # output rows per core
PX = R * W                 # 1600 real pixels
NT = 13                    # px tiles of 128
PXP = NT * 128             # 1664 padded pixels
NENT = H * W               # gather source entries per agent
NIDX = 2 * PXP // 16       # idx columns per pair (16-wrapped)
HCHUNKS = [(0, 832, [(0, 512), (512, 320)]), (832, 832, [(0, 512), (512, 320)])]
# u transposes write 128-wide blocks; chunks must be tile-aligned
UCHUNKS = [(0, 512), (512, 512), (1024, 512), (1536, 128)]
MCHUNKS = [(0, 832, [(0, 512), (512, 320)]), (832, 768, [(0, 512), (512, 256)])]


def _wrap_idx(idx_flat):
    """[N] -> [128, N//16] wrapped-in-16-partitions, replicated to 8 groups."""
    n = idx_flat.shape[0]
    w = idx_flat.reshape(n // 16, 16).T  # [16, N//16]
    return np.tile(w, (8, 1)).astype(np.int16)


def _host_warp_prep(theta, h0):
    """Per-(unit) gather indices + lerp scalars for output rows [h0,h0+R)."""
    ys = np.linspace(-1.0, 1.0, H, dtype=np.float32)[h0 : h0 + R]
    xs = np.linspace(-1.0, 1.0, W, dtype=np.float32)
    gx, gy = np.meshgrid(xs, ys)  # [R, W]
    sx = theta[0, 0] * gx + theta[0, 1] * gy + theta[0, 2]
    sy = theta[1, 0] * gx + theta[1, 1] * gy + theta[1, 2]
    px = (sx + 1.0) * (W - 1) / 2.0
    py = (sy + 1.0) * (H - 1) / 2.0
    x0 = np.floor(px).astype(np.int64)
    y0 = np.floor(py).astype(np.int64)
    fx = (px - x0).astype(np.float32)
    fy = (py - y0).astype(np.float32)

    scale = np.ones_like(fx)
    # x handling
    x0c = np.clip(x0, 0, W - 1)
    fxp = fx.copy()
    m = x0 == W - 1          # x1 out of bounds -> drop B/D taps
    fxp[m] = 0.0
    scale[m] *= 1.0 - fx[m]
    m = x0 == -1             # x0 out of bounds -> entry at x=0 is the B tap
    x0c[m] = 0
    fxp[m] = 0.0
    scale[m] *= fx[m]
    m = (x0 < -1) | (x0 > W - 1)
    x0c[m] = 0
    fxp[m] = 0.0
    scale[m] = 0.0
    # y handling (entry [y0] holds rows y0,y0+1; row 80 half is zeros)
    y0c = np.clip(y0, 0, H - 1)
    fyp = fy.copy()
    m = y0 == -1             # row0 is the F tap
    y0c[m] = 0
    fyp[m] = 0.0
    scale[m] *= fy[m]
    m = (y0 < -1) | (y0 > H - 1)
    y0c[m] = 0
    fyp[m] = 0.0
    scale[m] = 0.0

    idx = (y0c * W + x0c).reshape(-1)
    c0 = (scale * (1.0 - fyp)).reshape(-1)
    c1 = (scale * fyp).reshape(-1)
    fxp = fxp.reshape(-1)

    pad = PXP - PX
    idx = np.concatenate([idx, np.zeros(pad, np.int64)])
    fxp = np.concatenate([fxp, np.zeros(pad, np.float32)])
    c0 = np.concatenate([c0, np.zeros(pad, np.float32)])
    c1 = np.concatenate([c1, np.zeros(pad, np.float32)])
    return idx, fxp, c0, c1


def _host_warp_mask(mask_bj, theta, h0):
    """Bilinear warp of one [H,W] mask (zero padding) for rows [h0,h0+R)."""
    ys = np.linspace(-1.0, 1.0, H, dtype=np.float32)[h0 : h0 + R]
    xs = np.linspace(-1.0, 1.0, W, dtype=np.float32)
    gx, gy = np.meshgrid(xs, ys)
    sx = theta[0, 0] * gx + theta[0, 1] * gy + theta[0, 2]
    sy = theta[1, 0] * gx + theta[1, 1] * gy + theta[1, 2]
    px = (sx + 1.0) * (W - 1) / 2.0
    py = (sy + 1.0) * (H - 1) / 2.0
    x0 = np.floor(px).astype(np.int64)
    y0 = np.floor(py).astype(np.int64)
    wx = (px - x0).astype(np.float32)
    wy = (py - y0).astype(np.float32)

    def gat(xi, yi):
        inb = ((xi >= 0) & (xi < W) & (yi >= 0) & (yi < H)).astype(np.float32)
        v = mask_bj[np.clip(yi, 0, H - 1), np.clip(xi, 0, W - 1)]
        return v * inb

    out = (
        gat(x0, y0) * (1 - wx) * (1 - wy)
        + gat(x0 + 1, y0) * wx * (1 - wy)
        + gat(x0, y0 + 1) * (1 - wx) * wy
        + gat(x0 + 1, y0 + 1) * wx * wy
    )
    return out.reshape(-1)  # [PX]


def _layout(scene_of):
    """Scene starts/counts, ego set, non-ego pairs, conv groups, col perm."""
    nb = max(scene_of) + 1
    start = [None] * nb
    cnt = [0] * nb
    for a, b in enumerate(scene_of):
        if start[b] is None:
            start[b] = a
        cnt[b] += 1
    egos = [start[b] for b in range(nb)]
    non_ego = [j for j in range(len(scene_of)) if j not in egos]
    psz = int(os.environ.get("KERNEL_PAIRSZ", "2"))
    pairs = [tuple(non_ego[i : i + psz]) for i in range(0, len(non_ego), psz)]
    order = egos + non_ego
    groups = [order[i : i + 3] for i in range(0, len(order), 3)]
    col_of = {j: i for i, j in enumerate(order)}
    return start, cnt, egos, pairs, groups, col_of


def _runs(cols):
    """Split a sorted int list into (start, len) runs of consecutive ints."""
    runs = []
    for c in cols:
        if runs and c == runs[-1][0] + runs[-1][1]:
            runs[-1] = (runs[-1][0], runs[-1][1] + 1)
        else:
            runs.append((c, 1))
    return runs


def _ap(v, offset, dims):
    """Replace the free dims of AP v (keeping partition dim)."""
    return dataclasses.replace(
        v, offset=v.offset + offset, ap=[list(v.ap[0])] + [list(d) for d in dims])


def _build_program(nagents, scene_of, src_names):
    """Build the SPMD Bass program (identical for all cores)."""
    nc = bacc.Bacc("TRN2", target_bir_lowering=False, num_devices=NCORES,
                   dynamic_dma_scratch_size=16384)
    NA = nagents
    sstart, scnt, egos, pairs, groups, col_of = _layout(scene_of)
    npairs = len(pairs)

    psrc = [
        nc.dram_tensor(nm, [2 * (NENT + 1), 2 * C], F16, kind="ExternalInput")
        for nm in src_names
    ]
    idx_all = nc.dram_tensor("idx_all", [128, npairs * NIDX], I16,
                             kind="ExternalInput")
    scal_all = nc.dram_tensor("scal_all", [128, npairs * NIDX], F16,
                              kind="ExternalInput")
    ego_all = nc.dram_tensor("ego_all", [C, B * PXP], F16, kind="ExternalInput")
    ego_pmd = nc.dram_tensor("ego_pm", [128, B * NT * C], F16,
                             kind="ExternalInput")
    cmb = nc.dram_tensor("cmb", [128, NT * 2 * NA], F16, kind="ExternalInput")
    w1t1 = nc.dram_tensor("w1t1", [2 * C, 2 * C], F16, kind="ExternalInput")
    w1t2 = nc.dram_tensor("w1t2", [C, 2 * C], F16, kind="ExternalInput")
    w1t3 = nc.dram_tensor("w1t3", [C, 2 * C], F16, kind="ExternalInput")
    w2 = nc.dram_tensor("w2", [2 * C, 32], F16, kind="ExternalInput")
    bd3 = nc.dram_tensor("bd3", [96, 96], F16, kind="ExternalInput")
    bd4 = nc.dram_tensor("bd4", [96, 3], F16, kind="ExternalInput")
    mlpw65 = nc.dram_tensor("mlpw65", [C + 1, C], F16, kind="ExternalInput")
    sb = nc.dram_tensor("sb", [128, 6], F32, kind="ExternalInput")
    cb4v = nc.dram_tensor("cb4v", [128, 1], F32, kind="ExternalInput")
    sb2 = nc.dram_tensor("sb2", [96, 1], F32, kind="ExternalInput")
    sb3 = nc.dram_tensor("sb3", [96, 1], F32, kind="ExternalInput")
    ident = nc.dram_tensor("ident", [128, 128], F16, kind="ExternalInput")
    out = nc.dram_tensor("out", [B * C, PX], F32, kind="ExternalOutput")

    with TileContext(nc) as tc:
        with (
            tc.tile_pool(name="const", bufs=1) as cpool,
            tc.tile_pool(name="zs", bufs=1) as zpool,
            tc.tile_pool(name="work", bufs=2) as wpool,
            tc.tile_pool(name="att", bufs=1) as apool,
            tc.tile_pool(name="pmm", bufs=1, space="PSUM") as pmm,
            tc.tile_pool(name="ptr", bufs=2, space="PSUM") as ptr,
        ):
            # ---- constants ----
            idx_t = cpool.tile([128, npairs * NIDX], I16)
            nc.sync.dma_start(idx_t[:], idx_all[:, :])
            scal_t = cpool.tile([128, npairs * NIDX], F16)
            nc.sync.dma_start(scal_t[:], scal_all[:, :])
            ego_t = cpool.tile([C, B * PXP], F16)
            nc.sync.dma_start(ego_t[:], ego_all[:, :])
            ego_pm = cpool.tile([128, B * NT, C], F16)
            nc.sync.dma_start(ego_pm[:], ego_pmd[:, :].rearrange(
                "p (t c) -> p t c", c=C))
            cmb_t = cpool.tile([128, NT, 2 * NA], F16)
            nc.sync.dma_start(cmb_t[:], cmb[:, :].rearrange(
                "p (t a) -> p t a", a=2 * NA))
            t1w = cpool.tile([2 * C, 2 * C], F16)
            nc.sync.dma_start(t1w[:], w1t1[:, :])
            t2w = cpool.tile([C, 2 * C], F16)
            nc.sync.dma_start(t2w[:], w1t2[:, :])
            t3w = cpool.tile([C, 2 * C], F16)
            nc.sync.dma_start(t3w[:], w1t3[:, :])
            w2_t = cpool.tile([2 * C, 32], F16)
            nc.sync.dma_start(w2_t[:], w2[:, :])
            bd3_t = cpool.tile([96, 96], F16)
            nc.sync.dma_start(bd3_t[:], bd3[:, :])
            bd4_t = cpool.tile([96, 3], F16)
            nc.sync.dma_start(bd4_t[:], bd4[:, :])
            mlpw_t = cpool.tile([C + 1, C], F16)
            nc.sync.dma_start(mlpw_t[:], mlpw65[:, :])
            sb_t = cpool.tile([128, 6], F32)
            nc.sync.dma_start(sb_t[:], sb[:, :])
            cb4_t = cpool.tile([128, 1], F32)
            nc.sync.dma_start(cb4_t[:], cb4v[:, :])
            sb2_t = cpool.tile([96, 1], F32)
            nc.sync.dma_start(sb2_t[:], sb2[:, :])
            sb3_t = cpool.tile([96, 1], F32)
            nc.sync.dma_start(sb3_t[:], sb3[:, :])
            id_t = cpool.tile([128, 128], F16)
            nc.sync.dma_start(id_t[:], ident[:, :])

            # channel-major pair z tiles (rows = a*64+c), px-major nbr tiles
            zp_all = [zpool.tile([128, PXP], F16, name=f"zp{p}", tag=f"zp{p}")
                      for p in range(npairs)]
            nbrp_all = [zpool.tile([128, 2 * NT, C], F16, name=f"nbp{p}",
                                   tag=f"nbp{p}")
                        for p in range(npairs)]
            h1_all = {}
            # s (pixel-major) accumulates from the fused conv4+transpose mms
            s_ps = pmm.tile([128, NT, 16], F32, tag="s_ps", bufs=1)

            def conv1_ego(j):
                b = scene_of[j]
                h1_j = wpool.tile([128, PXP], F16, name=f"h1_{j}",
                                  tag=f"h1_{j}", bufs=1)
                h1_all[j] = h1_j
                for (o, n, mms) in HCHUNKS:
                    p1 = pmm.tile([128, 832], F32, tag="p34", bufs=2)
                    for (mo, mn) in mms:
                        nc.tensor.matmul(
                            p1[:, mo : mo + mn], t3w[:],
                            ego_t[:, b * PXP + o + mo : b * PXP + o + mo + mn],
                            start=True, stop=True)
                    nc.scalar.activation(h1_j[:, o : o + n], p1[:, 0:n],
                                         Act.Relu, bias=sb_t[:, 1:2], scale=1.0)

            def conv1_pair(j, p, a):
                b = scene_of[j]
                h1_j = wpool.tile([128, PXP], F16, name=f"h1_{j}",
                                  tag=f"h1_{j}", bufs=1)
                h1_all[j] = h1_j
                zp = zp_all[p]
                for (o, n, mms) in HCHUNKS:
                    p1 = pmm.tile([128, 832], F32, tag="p34", bufs=2)
                    for (mo, mn) in mms:
                        nc.tensor.matmul(
                            p1[:, mo : mo + mn],
                            t1w[C * a : C * a + C, :],
                            zp[C * a : C * a + C, o + mo : o + mo + mn],
                            start=True, stop=False)
                        nc.tensor.matmul(
                            p1[:, mo : mo + mn], t2w[:],
                            ego_t[:, b * PXP + o + mo : b * PXP + o + mo + mn],
                            start=False, stop=True)
                    nc.scalar.activation(h1_j[:, o : o + n], p1[:, 0:n],
                                         Act.Relu, bias=sb_t[:, 1:2], scale=1.0)

            def conv234(g):
                grp = groups[g]
                ng = len(grp)
                hs2 = wpool.tile([96, PXP], F16, tag="hs2", bufs=1)
                hs3 = wpool.tile([96, PXP], F16, tag="hs3", bufs=1)
                for (o, n, mms) in HCHUNKS:
                    sl = slice(o, o + n)
                    ph2 = pmm.tile([96, 832], F32, tag="p34", bufs=2)
                    for q, jj in enumerate(grp):
                        for (mo, mn) in mms:
                            nc.tensor.matmul(
                                ph2[32 * q : 32 * q + 32, mo : mo + mn],
                                w2_t[:],
                                h1_all[jj][:, o + mo : o + mo + mn],
                                start=True, stop=True)
                    nc.scalar.activation(hs2[0 : 32 * ng, sl],
                                         ph2[0 : 32 * ng, 0:n], Act.Relu,
                                         bias=sb2_t[0 : 32 * ng, 0:1],
                                         scale=1.0)
                    p34 = pmm.tile([96, 832], F32, tag="p34", bufs=2)
                    for (mo, mn) in mms:
                        nc.tensor.matmul(
                            p34[0 : 32 * ng, mo : mo + mn],
                            bd3_t[0 : 32 * ng, 0 : 32 * ng],
                            hs2[0 : 32 * ng, o + mo : o + mo + mn],
                            start=True, stop=True)
                    nc.scalar.activation(hs3[0 : 32 * ng, sl],
                                         p34[0 : 32 * ng, 0:n], Act.Relu,
                                         bias=sb3_t[0 : 32 * ng, 0:1],
                                         scale=1.0)
                # conv4 fused with the s transpose: per px tile,
                # s_pm[px, 3g+q] = sum_c w4[c] * h3_q[32q+c, px]
                for t in range(NT):
                    nc.tensor.matmul(
                        s_ps[:, t, 3 * g : 3 * g + ng],
                        hs3[0 : 32 * ng, 128 * t : 128 * (t + 1)],
                        bd4_t[0 : 32 * ng, 0:ng],
                        start=True, stop=True)

            # ---- ego agents: direct channel-major features, conv1 early ----
            for j in egos:
                conv1_ego(j)
            conv234(0)  # group 0 = the ego agents

            # ---- non-ego pairs: gather + lerp + transpose + conv1 ----
            done_h1 = set(egos)
            done_groups = {0}
            for p, pr in enumerate(pairs):
                na2 = len(pr)  # 2, or 1 for a trailing single
                nblk = 2 * NT if na2 == 2 else NT
                # gather: blocks are (tile, agent) interleaved with indices
                # pre-offset for agent 1; split into 2 chunks to stay under
                # the SWDGE descriptor ring size
                g_t = wpool.tile([128, nblk, 4 * C], F16, tag="g", bufs=2)
                src_flat = psrc[p][:, :].rearrange("a b -> (a b)")
                src_win = dataclasses.replace(
                    src_flat, ap=[[2 * C, 2 * (NENT + 1) - 1], [1, 4 * C]]
                )
                gchunks = [(i, min(7, nblk - i)) for i in range(0, nblk, 7)]
                for (b0, bn) in gchunks:
                    nc.gpsimd.dma_gather(
                        g_t[:, b0 : b0 + bn, :],
                        src_win,
                        idx_t[:, p * NIDX + b0 * 8 :
                              p * NIDX + (b0 + bn) * 8],
                        num_idxs=bn * 128,
                        num_idxs_reg=bn * 128,
                        elem_size=4 * C,
                        elem_step=2 * C,
                        single_packet=False,
                    )
                # ---- bilinear combine: nbr = w00*A+w10*C + w01*B+w11*D ----
                t1_t = wpool.tile([128, nblk, 2 * C], F16, tag="t1", bufs=2)
                t2_t = wpool.tile([128, nblk, 2 * C], F16, tag="t2", bufs=2)
                nbr_t = nbrp_all[p]
                wq = scal_t[:, p * NIDX : (p + 1) * NIDX]
                for q, dst in ((0, t1_t[:, :, 0:C]), (1, t1_t[:, :, C : 2 * C]),
                               (2, t2_t[:, :, 0:C]), (3, t2_t[:, :, C : 2 * C])):
                    w_ap = dataclasses.replace(
                        wq, offset=wq.offset + 2 * q,
                        ap=[list(wq.ap[0]), [8, nblk], [0, C // 2], [1, 2]])
                    src = g_t[:, :, q * C : (q + 1) * C]
                    nc.vector.tensor_tensor(
                        dst.rearrange("p a (c d) -> p a c d", d=2),
                        src.rearrange("p a (c d) -> p a c d", d=2),
                        w_ap, Alu.mult)
                nc.vector.tensor_tensor(t1_t[:, :, 0:C], t1_t[:, :, 0:C],
                                        t2_t[:, :, 0:C], Alu.add)
                nc.vector.tensor_tensor(t1_t[:, :, C : 2 * C],
                                        t1_t[:, :, C : 2 * C],
                                        t2_t[:, :, C : 2 * C], Alu.add)
                nc.vector.tensor_tensor(
                    nbr_t[:, 0:nblk, :], t1_t[:, :, 0:C],
                    t1_t[:, :, C : 2 * C], Alu.add)
                # ---- transpose px-major -> channel-major into zpair ----
                # each px tile transposes BOTH agents' channels at once
                zp = zp_all[p]
                nv = nbr_t[:]
                for t0 in range(0, NT, 4):
                    tn = min(4, NT - t0)
                    tr_ps = ptr.tile([128, 512], F16, tag="tr")
                    for t in range(t0, t0 + tn):
                        if na2 == 2:
                            src_t = _ap(nv, 2 * t * C, [[1, 2 * C]])
                        else:
                            src_t = _ap(nv, t * C, [[1, C]])
                        nc.tensor.transpose(
                            tr_ps[0 : 64 * na2,
                                  128 * (t - t0) : 128 * (t - t0 + 1)],
                            src_t, id_t[:])
                    nc.scalar.activation(
                        zp[0 : 64 * na2, 128 * t0 : 128 * (t0 + tn)],
                        tr_ps[0 : 64 * na2, 0 : 128 * tn], Act.Copy)
                # ---- conv1 for the pair's agents ----
                for a, j in enumerate(pr):
                    conv1_pair(j, p, a)
                    done_h1.add(j)
                for g in range(len(groups)):
                    if g not in done_groups and all(
                            jj in done_h1 for jj in groups[g]):
                        conv234(g)
                        done_groups.add(g)

            # ---- attention in pixel-major ----
            # e = exp(relu(s_raw + cb4)) = max(exp(s_raw + cb4), 1)
            e_t = apool.tile([128, NT, NA], F16)
            nc.scalar.activation(e_t[:], s_ps[:, :, 0:NA], Act.Exp,
                                 bias=cb4_t[:, 0:1], scale=1.0)
            nc.vector.tensor_scalar_max(e_t[:], e_t[:], 1.0)
            # ep = e * (cm != 0); al = e * cm   (cm columns are s-col order)
            ep_t = apool.tile([128, NT, NA], F16)
            nc.vector.tensor_tensor(ep_t[:], e_t[:], cmb_t[:, :, NA : 2 * NA],
                                    Alu.mult)
            al_t = apool.tile([128, NT, NA], F16)
            nc.vector.tensor_tensor(al_t[:], e_t[:], cmb_t[:, :, 0:NA],
                                    Alu.mult)
            # den per scene (chain adds over the scene's agent columns)
            den_t = apool.tile([128, NT, B], F16)
            for b in range(B):
                cols = sorted(col_of[j]
                              for j in range(sstart[b], sstart[b] + scnt[b]))
                nc.vector.tensor_tensor(
                    den_t[:, :, b : b + 1], ep_t[:, :, cols[0] : cols[0] + 1],
                    ep_t[:, :, cols[1] : cols[1] + 1], Alu.add)
                for ck in cols[2:]:
                    nc.vector.tensor_tensor(
                        den_t[:, :, b : b + 1], den_t[:, :, b : b + 1],
                        ep_t[:, :, ck : ck + 1], Alu.add)
            rec_t = apool.tile([128, NT, B], F16)
            with nc.allow_low_precision(reason="den>=1, fp16 rec ok"):
                nc.vector.reciprocal(rec_t[:], den_t[:])
            # alpha = al * rec[scene]
            alp_t = apool.tile([128, NT, NA], F16)
            for b in range(B):
                cols = sorted(col_of[j]
                              for j in range(sstart[b], sstart[b] + scnt[b]))
                for (c0, nj) in _runs(cols):
                    r_ap = _ap(rec_t[:], b, [[B, NT], [0, nj]])
                    nc.vector.tensor_tensor(alp_t[:, :, c0 : c0 + nj],
                                            al_t[:, :, c0 : c0 + nj], r_ap,
                                            Alu.mult)
            # scaled_j = alpha_j * nbr_j (pixel-major); one op per pair
            scl_ego = [apool.tile([128, NT, C], F16, name=f"sce{b}")
                       for b in range(B)]
            scl_pr = [apool.tile([128, 2 * NT, C], F16, name=f"scp{p}")
                      for p in range(npairs)]
            for b in range(B):
                j = egos[b]
                a_ap = _ap(alp_t[:], col_of[j], [[NA, NT], [0, C]])
                nc.vector.tensor_tensor(
                    scl_ego[b][:], ego_pm[:, b * NT : (b + 1) * NT, :], a_ap,
                    Alu.mult)
            for p, pr in enumerate(pairs):
                na2 = len(pr)
                cols = [col_of[j] for j in pr]
                dc = cols[1] - cols[0] if na2 == 2 else 1
                a_ap = _ap(alp_t[:], cols[0], [[NA, NT], [dc, na2], [0, C]])
                nc.vector.tensor_tensor(
                    scl_pr[p][:, 0 : na2 * NT, :].rearrange(
                        "p (t a) c -> p t a c", a=na2),
                    nbrp_all[p][:, 0 : na2 * NT, :].rearrange(
                        "p (t a) c -> p t a c", a=na2),
                    a_ap, Alu.mult)
            # fold per scene
            u_pm = [apool.tile([128, NT, C], F16, name=f"upm{b}")
                    for b in range(B)]
            slices = {}  # agent j -> AP of its scaled px-major tile
            for b in range(B):
                slices[egos[b]] = scl_ego[b][:]
            for p, pr in enumerate(pairs):
                na2 = len(pr)
                for a, j in enumerate(pr):
                    slices[j] = _ap(scl_pr[p][:], (a * C) if na2 == 2 else 0,
                                    [[na2 * C, NT], [1, C]])
            for b in range(B):
                js = list(range(sstart[b], sstart[b] + scnt[b]))
                nc.vector.tensor_tensor(u_pm[b][:], slices[js[0]],
                                        slices[js[1]], Alu.add)
                for jk in js[2:]:
                    nc.vector.tensor_tensor(u_pm[b][:], u_pm[b][:],
                                            slices[jk], Alu.add)
            # ---- transpose u back to channel-major, MLP, write out ----
            for b in range(B):
                u_sb = apool.tile([C + 1, PXP], F16, name=f"usb{b}")
                nc.gpsimd.memset(u_sb[C : C + 1, :], 1.0)
                for (o, n) in UCHUNKS:
                    u_ps = ptr.tile([C, 512], F16, tag="tr")
                    for t in range(o // 128, (o + n) // 128):
                        nc.tensor.transpose(
                            u_ps[:, 128 * t - o : 128 * (t + 1) - o],
                            u_pm[b][:, t, :], id_t[:])
                    nc.vector.tensor_scalar(u_sb[0:C, o : o + n],
                                            u_ps[:, 0:n], 0.0, None, Alu.add)
                for (o, n, mms) in MCHUNKS:
                    mps = pmm.tile([C, 832], F32, tag="p34", bufs=2)
                    for (mo, mn) in mms:
                        nc.tensor.matmul(mps[:, mo : mo + mn], mlpw_t[:],
                                         u_sb[:, o + mo : o + mo + mn],
                                         start=True, stop=True)
                    ob = wpool.tile([C, 832], F32, tag="ob")
                    nc.scalar.activation(ob[:, 0:n], mps[:, 0:n], Act.Copy)
                    nc.sync.dma_start(out[b * C : (b + 1) * C, o : o + n],
                                      ob[:, 0:n])

    nc.compile()
    return nc


_PROG_CACHE = {}
_LAST_RES = None


def kernel(**inputs):
    x = np.asarray(inputs["x"], np.float32)
    mask = np.asarray(inputs["mask"], np.float32)
    record_len = np.asarray(inputs["record_len"])
    ptm = np.asarray(inputs["pairwise_t_matrix"], np.float32)
    rec = [int(v) for v in record_len]
    agents = [(b, j) for b in range(B) for j in range(rec[b])]
    nagents = len(agents)
    scene_of = [b for (b, j) in agents]
    NA = nagents
    sstart, scnt, egos, pairs, groups, col_of = _layout(scene_of)
    npairs = len(pairs)

    # ---- regroup x into per-scene node features ----
    node = np.zeros((B, L, C, H, W), np.float32)
    idx0 = 0
    for b, n in enumerate(rec):
        node[b, :n] = x[idx0 : idx0 + n]
        idx0 += n

    # ---- gather sources (pairs): dup-row pixel-major fp16, concatenated ----
    def agent_src(a):
        b, j = agents[a]
        feat = node[b, j]  # [C, H, W]
        ent = np.zeros((H + 1, W, 2 * C), np.float16)
        pm = feat.transpose(1, 2, 0).astype(np.float16)  # [H, W, C]
        ent[:H, :, :C] = pm
        ent[:H - 1, :, C:] = pm[1:]
        arr = np.zeros((NENT + 1, 2 * C), np.float16)
        arr[:NENT] = ent[:H].reshape(NENT, 2 * C)
        return arr

    src_names = [f"psrc{p}" for p in range(npairs)]
    src_arrs = {}
    for p, pr in enumerate(pairs):
        arr = np.zeros((2 * (NENT + 1), 2 * C), np.float16)
        for a, j in enumerate(pr):
            arr[a * (NENT + 1) : a * (NENT + 1) + NENT + 1] = agent_src(j)
        src_arrs[src_names[p]] = arr

    # ---- per-core index/scalar/mask/ego prep ----
    per_core = []
    for k in range(NCORES):
        h0 = k * R
        idx_cols = np.zeros((128, npairs * NIDX), np.int16)
        scal_cols = np.zeros((128, npairs * NIDX), np.float16)
        cmb_arr = np.zeros((128, NT * 2 * NA), np.float16)
        ego_arr = np.zeros((C, B * PXP), np.float16)
        ego_pm_arr = np.zeros((128, B * NT * C), np.float16)
        for b in range(B):
            ego = np.zeros((C, PXP), np.float16)
            ego[:, :PX] = node[b, 0][:, h0 : h0 + R, :].reshape(C, PX)
            ego_arr[:, b * PXP : (b + 1) * PXP] = ego
            # px-major: [PXP, C] -> [NT, 128, C] -> [128, NT*C]
            epm = ego.T.reshape(NT, 128, C).transpose(1, 0, 2)
            ego_pm_arr[:, b * NT * C : (b + 1) * NT * C] = epm.reshape(
                128, NT * C)
        for p, pr in enumerate(pairs):
            na2 = len(pr)
            nblk = 2 * NT if na2 == 2 else NT
            gidx = np.zeros((nblk * 128,), np.int64)
            for a, j in enumerate(pr):
                b, jj = agents[j]
                theta = ptm[b, jj, 0]
                idx, fxp, c0, c1 = _host_warp_prep(theta, h0)
                gi = gidx.reshape(NT, na2, 128)
                gi[:, a, :] = (idx + a * (NENT + 1)).reshape(NT, 128)
                w00 = (c0 * (1.0 - fxp)).astype(np.float16)
                w10 = (c1 * (1.0 - fxp)).astype(np.float16)
                w01 = (c0 * fxp).astype(np.float16)
                w11 = (c1 * fxp).astype(np.float16)
                sc = scal_cols[:, p * NIDX : (p + 1) * NIDX]
                for t in range(NT):
                    pxs = slice(128 * t, 128 * (t + 1))
                    blk = na2 * t + a
                    for q, wv in enumerate((w00, w10, w01, w11)):
                        sc[:, 8 * blk + 2 * q] = wv[pxs]
                        sc[:, 8 * blk + 2 * q + 1] = wv[pxs]
            idx_cols[:, p * NIDX : p * NIDX + nblk * 8] = _wrap_idx(gidx)
        for a, (b, j) in enumerate(agents):
            theta = ptm[b, j, 0]
            col = col_of[a]
            wm = _host_warp_mask(mask[b, j], theta, h0)
            wmp = np.zeros(PXP, np.float32)
            wmp[:PX] = wm
            wmz = (wmp != 0).astype(np.float32)
            wmz[PX:] = 1.0  # keep den >= 1 on padded pixels
            cm_pm = wmp.reshape(NT, 128).T.astype(np.float16)   # [128, NT]
            cmz_pm = wmz.reshape(NT, 128).T.astype(np.float16)
            for t in range(NT):
                cmb_arr[:, t * 2 * NA + col] = cm_pm[:, t]
                cmb_arr[:, t * 2 * NA + NA + col] = cmz_pm[:, t]
        per_core.append((idx_cols, scal_cols, cmb_arr, ego_arr, ego_pm_arr))

    # ---- shared small tensors ----
    def gf(n):
        return np.asarray(inputs[n], np.float32)

    sb = np.zeros((128, 6), np.float32)
    sb2v = np.zeros((96, 1), np.float32)
    sb3v = np.zeros((96, 1), np.float32)
    a1 = gf("g1") / np.sqrt(gf("rv1") + EPS)
    sb[:, 1] = gf("be1") + (gf("cb1") - gf("rm1")) * a1
    a2 = gf("g2") / np.sqrt(gf("rv2") + EPS)
    b2f = gf("be2") + (gf("cb2") - gf("rm2")) * a2
    a3 = gf("g3") / np.sqrt(gf("rv3") + EPS)
    b3f = gf("be3") + (gf("cb3") - gf("rm3")) * a3
    for q in range(3):
        sb2v[32 * q : 32 * q + 32, 0] = b2f
        sb3v[32 * q : 32 * q + 8, 0] = b3f

    w1f = (gf("w1") * a1[None, :]).astype(np.float16)  # [128, 128]
    w1n = w1f[0:C]
    w1e = w1f[C : 2 * C]
    w3f = (gf("w3") * a3[None, :]).astype(np.float16)  # [32, 8]
    w4f = gf("w4").astype(np.float16)                  # [8, 1]
    bd3a = np.zeros((96, 96), np.float16)
    bd4a = np.zeros((96, 3), np.float16)
    for q in range(3):
        bd3a[32 * q : 32 * q + 32, 32 * q : 32 * q + 8] = w3f
        bd4a[32 * q : 32 * q + 8, q] = w4f[:, 0]

    mlp65 = np.zeros((C + 1, C), np.float16)
    mlp65[:C] = gf("mlp_w").astype(np.float16)
    mlp65[C] = gf("mlp_b").astype(np.float16)

    shared = {
        "w1t1": np.concatenate([w1n, w1n], axis=0),
        "w1t2": w1e,
        "w1t3": w1n + w1e,
        "w2": (gf("w2") * a2[None, :]).astype(np.float16),
        "bd3": bd3a,
        "bd4": bd4a,
        "mlpw65": mlp65,
        "sb": sb,
        "sb2": sb2v,
        "sb3": sb3v,
        "cb4v": np.full((128, 1), gf("cb4")[0], np.float32),
        "ident": np.eye(128, dtype=np.float16),
    }
    shared.update(src_arrs)

    key = (nagents, tuple(scene_of))
    if key not in _PROG_CACHE:
        _PROG_CACHE[key] = _build_program(nagents, scene_of, src_names)
    nc = _PROG_CACHE[key]

    in_maps = []
    for k in range(NCORES):
        idx_cols, scal_cols, cmb_arr, ego_arr, ego_pm_arr = per_core[k]
        m = dict(shared)
        m["idx_all"] = idx_cols
        m["scal_all"] = scal_cols
        m["cmb"] = cmb_arr
        m["ego_all"] = ego_arr
        m["ego_pm"] = ego_pm_arr
        in_maps.append(m)

    trace = bool(os.environ.get("KERNEL_TRACE"))
    res = run_bass_kernel_spmd(nc, in_maps, core_ids=list(range(NCORES)),
                               trace=trace)
    global _LAST_RES
    _LAST_RES = res

    out = np.zeros((B, C, H, W), np.float32)
    for k in range(NCORES):
        o = res.results[k]["out"]  # [B*C, PX]
        out[:, :, k * R : (k + 1) * R, :] = o.reshape(B, C, R, W)
    return out
